# revision 1
# baseline (speedup 1.0000x reference)
"""Trainium2 Bass kernel for nn_Decoder_55688545960558.

Hierarchical-attention GRU decoder step:
  word-level Bahdanau attention over (B,T,S,D) encoder outputs,
  masked GRU scan over T turns, utterance-level Bahdanau attention,
  decoder GRU step on [ctx_vec ; embed(x)].

Sharding: pure data-parallel over batch (64 -> 8 per core), no
collectives (the module is independent per batch element). All heavy
matmuls run in bf16 with fp32 PSUM accumulation; elementwise gate math
in fp32. Activations are kept "transposed" (feature dim on partitions,
batch/time on the free dim) end-to-end so every matmul contraction
lands on the partition axis with zero on-device transposes; the single
output transpose at the end goes through the PE with an identity.

SBUF is tag-rotated across stages: the ctx_kernel slot is reused for
the decoder kernel's first half, ctx_rec_kernel's for the second half,
the word-attention weights' for the utterance-attention weights, and
the encoder stream buffers for the streamed dec_rec_kernel chunks.
"""

from contextlib import ExitStack

import numpy as np
import ml_dtypes

import concourse.bass as bass
import concourse.mybir as mybir
import concourse.tile as tile
from concourse import bacc
from concourse.bass_utils import run_bass_kernel_spmd

F32 = mybir.dt.float32
BF16 = mybir.dt.bfloat16
AF = mybir.ActivationFunctionType
OP = mybir.AluOpType
AX = mybir.AxisListType

NCORES = 8
B = 64            # global batch
BL = B // NCORES  # batches per core (8)
T = 10
S = 50
R = T * S         # 500
D = 1024
U = 1024
C = D // 128      # 8 chunks of the feature dim
G3 = 3 * D        # 3072
KD = (D + D) // 128  # 16 contraction chunks for decoder input (EMB + D)

DEBUG = False     # adds intermediate DRAM outputs for bring-up


def _bcast_mid(ap, n):
    """Insert a 0-stride broadcast dim of size n as dim 1 (after partitions)."""
    return bass.AP(tensor=ap.tensor, offset=ap.offset,
                   ap=[ap.ap[0], [0, n]] + list(ap.ap[1:]))


def _bcast_last(ap, n):
    """Append a 0-stride broadcast dim of size n as the innermost dim."""
    return bass.AP(tensor=ap.tensor, offset=ap.offset,
                   ap=list(ap.ap) + [[0, n]])


def build():
    nc = bacc.Bacc("TRN2", target_bir_lowering=False, debug=False,
                   num_devices=NCORES)

    def din(name, shape, dt):
        return nc.dram_tensor(name, list(shape), dt, kind="ExternalInput").ap()

    ins = {}
    ins["enc"] = din("enc_t", [BL, 128, C, R], BF16)
    ins["hidT_f"] = din("hidT_f", [128, C, BL], F32)
    ins["hidT_b"] = din("hidT_b", [128, C, BL], BF16)
    ins["xembT_b"] = din("xembT_b", [128, C, BL], BF16)
    ins["w1w"] = din("w1w", [128, C, U], BF16)
    ins["w2w"] = din("w2w", [128, C, U], BF16)
    ins["vw"] = din("vw_rep", [128, C, 128], BF16)   # V bcast across columns
    ins["w1u"] = din("w1u", [128, C, U], BF16)
    ins["w2u"] = din("w2u", [128, C, U], BF16)
    ins["vu"] = din("vu_rep", [128, C, 128], BF16)
    ins["ctxk"] = din("ctxk", [128, C, G3], BF16)
    ins["ctxrk"] = din("ctxrk", [128, C, G3], BF16)
    ins["deckA"] = din("deckA", [128, C, G3], BF16)     # dec_kernel rows 0:1024
    ins["deckB"] = din("deckB", [128, C, G3], BF16)     # dec_kernel rows 1024:2048
    ins["decrk"] = din("decrk", [C, 128, G3], BF16)     # chunk-major for streaming
    ins["qb_w"] = din("qb_w", [128, C], F32)
    ins["qb_u"] = din("qb_u", [128, C], F32)
    ins["cbx"] = din("cbx", [128, 3 * C], F32)
    ins["cb1h"] = din("cb1h_b", [1, D], BF16)
    ins["dbx"] = din("dbx", [128, 3 * C], F32)
    ins["db1h"] = din("db1h_b", [1, D], BF16)
    ins["mask"] = din("mask_t", [128, T, BL], F32)   # pre-scaled by -0.5
    ins["ones"] = din("ones_b", [1, BL], BF16)
    ins["ident"] = din("ident", [128, 128], F32)

    ins["out"] = nc.dram_tensor("out", [BL, D], F32, kind="ExternalOutput").ap()
    dbg = {}
    if DEBUG:
        dbg["q"] = nc.dram_tensor("dbg_q", [128, C, BL], F32,
                                  kind="ExternalOutput").ap()
        dbg["ctxT"] = nc.dram_tensor("dbg_ctxT", [128, C, BL, T], F32,
                                     kind="ExternalOutput").ap()
        dbg["seq"] = nc.dram_tensor("dbg_seq", [128, C, BL, T], F32,
                                    kind="ExternalOutput").ap()
        dbg["ctxv"] = nc.dram_tensor("dbg_ctxv", [128, C, BL], F32,
                                     kind="ExternalOutput").ap()
        dbg["xmd"] = nc.dram_tensor("dbg_xmd", [128, 3, C, BL], F32,
                                    kind="ExternalOutput").ap()
        dbg["hmd"] = nc.dram_tensor("dbg_hmd", [128, 3, C, BL], F32,
                                    kind="ExternalOutput").ap()
        dbg["stT"] = nc.dram_tensor("dbg_stT", [128, C, BL], F32,
                                    kind="ExternalOutput").ap()
    ins["dbg"] = dbg

    with tile.TileContext(nc) as tc:
        _emit(nc, tc, ins)
    nc.compile()
    return nc


def _emit(nc, tc, ins):
    dbg = ins["dbg"]
    es = ExitStack()

    pers = es.enter_context(tc.tile_pool(name="pers", bufs=1))
    # big rotating weight slots (tag-shared across stages)
    slot1 = es.enter_context(tc.tile_pool(name="slot1", bufs=1))   # ctxk -> deckA
    slot2 = es.enter_context(tc.tile_pool(name="slot2", bufs=1))   # ctxrk -> deckB
    wsmall = es.enter_context(tc.tile_pool(name="wsmall", bufs=1))  # word -> utt
    streamp = es.enter_context(tc.tile_pool(name="streamp", bufs=3))  # enc / decrk
    tanhp = es.enter_context(tc.tile_pool(name="tanhp", bufs=9))
    s1small = es.enter_context(tc.tile_pool(name="s1small", bufs=2))
    prodp = es.enter_context(tc.tile_pool(name="prodp", bufs=3))
    gtmp = es.enter_context(tc.tile_pool(name="gtmp", bufs=3))
    hstate = es.enter_context(tc.tile_pool(name="hstate", bufs=2))
    su_pool = es.enter_context(tc.tile_pool(name="su_pool", bufs=9))
    s4tmp = es.enter_context(tc.tile_pool(name="s4tmp", bufs=1))

    def ld(pool, dram_ap, shape, dt, name, engine=None, chunked=False):
        t = pool.tile(list(shape), dt, tag=name, name=name)
        eng = engine or nc.sync
        if chunked:
            for k in range(shape[1]):
                eng.dma_start(out=t[:, k], in_=dram_ap[:, k])
        else:
            eng.dma_start(out=t[:], in_=dram_ap)
        return t

    # ---- critical-path DMA order on the sync queue:
    #      tiny q prerequisites, w1w, enc b0, w2w, enc b1.., then the rest
    hidT_b = ld(pers, ins["hidT_b"], [128, C, BL], BF16, "hidT_b")
    qbw_s = ld(pers, ins["qb_w"], [128, C], F32, "qbw")
    vw_s = ld(pers, ins["vw"], [128, C, 128], BF16, "vw")
    w1w_s = wsmall.tile([128, C, U], BF16, tag="wA", name="w1w")
    # enc batches and GRU weight chunks hand-interleaved on the sync queue:
    # enc batch b is needed at ~13.4us*b, the GRU weights only after stage 1
    ctxk_s = slot1.tile([128, C, G3], BF16, tag="slot1", name="ctxk")
    ctxrk_s = slot2.tile([128, C, G3], BF16, tag="slot2", name="ctxrk")
    gru_chunks = [(ctxk_s, ins["ctxk"], k) for k in range(C)] + \
                 [(ctxrk_s, ins["ctxrk"], k) for k in range(C)]
    gi = 0

    def gru_chunk_dma(n):
        nonlocal gi
        for _ in range(n):
            if gi < len(gru_chunks):
                tl, dr, k = gru_chunks[gi]
                nc.sync.dma_start(out=tl[:, k], in_=dr[:, k])
                gi += 1

    enc_b0 = streamp.tile([128, C, R], BF16, tag="st", name="enc0")
    for k in range(C):
        # pairwise: stage-1's first matmul chain consumes (w1w[k], enc0[k])
        nc.sync.dma_start(out=w1w_s[:, k], in_=ins["w1w"][:, k])
        nc.sync.dma_start(out=enc_b0[:, k], in_=ins["enc"][0][:, k])
    w2w_s = ld(wsmall, ins["w2w"], [128, C, U], BF16, "wB")
    enc_tiles = [enc_b0,
                 ld(streamp, ins["enc"][1], [128, C, R], BF16, "st")]
    for b in range(2, BL):
        gru_chunk_dma(2)
        enc_tiles.append(ld(streamp, ins["enc"][b], [128, C, R], BF16, "st"))
    gru_chunk_dma(len(gru_chunks))
    hidT_f = ld(pers, ins["hidT_f"], [128, C, BL], F32, "hidT_f")
    xembT_b = ld(pers, ins["xembT_b"], [128, C, BL], BF16, "xembT_b")
    vu_s = ld(pers, ins["vu"], [128, C, 128], BF16, "vu")
    qbu_s = ld(pers, ins["qb_u"], [128, C], F32, "qbu")
    cbx_s = ld(pers, ins["cbx"], [128, 3 * C], F32, "cbx")
    cb1h_s = ld(pers, ins["cb1h"], [1, D], BF16, "cb1h")
    dbx_s = ld(pers, ins["dbx"], [128, 3 * C], F32, "dbx")
    db1h_s = ld(pers, ins["db1h"], [1, D], BF16, "db1h")
    mask_s = ld(pers, ins["mask"], [128, T, BL], F32, "mask")
    ones_s = ld(pers, ins["ones"], [1, BL], BF16, "ones")
    ident_s = ld(pers, ins["ident"], [128, 128], F32, "ident")

    # cross-stage activation tensors
    ctxT_f = pers.tile([128, C, BL, T], F32, tag="ctxT_f")
    ctxT_b = pers.tile([128, C, BL, T], BF16, tag="ctxT_b")
    seq_b = pers.tile([128, C, BL, T], BF16, tag="seq_b")
    ctxvT_f = pers.tile([128, C, BL], F32, tag="ctxvT_f")
    ctxvT_b = pers.tile([128, C, BL], BF16, tag="ctxvT_b")
    qu_s = pers.tile([128, C, BL], F32, tag="qu")
    qsb = pers.tile([128, C, BL], F32, tag="qsb")
    xg = [pers.tile([128, C, BL, T], BF16, tag=f"xg{g}", name=f"xg{g}")
          for g in range(3)]

    # =================== STAGE 1: word attention ===================
    with tc.tile_pool(name="ps_score", bufs=6, space="PSUM") as p_score, \
         tc.tile_pool(name="pq", bufs=1, space="PSUM") as pq, \
         tc.tile_pool(name="ps_sc", bufs=1, space="PSUM") as p_sc:
        def emit_score_mm(enc_b, m):
            ps = p_score.tile([128, R], F32, tag="ps")
            for k in range(C):
                nc.tensor.matmul(out=ps[:],
                                 lhsT=w1w_s[:, k, m * 128:(m + 1) * 128],
                                 rhs=enc_b[:, k], start=(k == 0),
                                 stop=(k == C - 1))
            return ps

        def emit_tanh(ps, m, b):
            th = tanhp.tile([128, R], BF16, tag="th")
            nc.scalar.activation(out=th[:], in_=ps[:], func=AF.Tanh,
                                 bias=qsb[:, m, b:b + 1])
            return th

        for b in range(BL):
            enc_b = enc_tiles[b]
            ths = []
            if b == 0:
                # b0: emit the first 4 score matmuls, THEN the queries (so
                # the PE reaches them as w2w's DMA lands, without any tanh
                # reading qsb before it is written), then their tanhs
                pss = [emit_score_mm(enc_b, m) for m in range(4)]
                p_q = pq.tile([128, C, BL], F32)
                for mm in range(C):
                    for k in range(C):
                        nc.tensor.matmul(out=p_q[:, mm],
                                         lhsT=w2w_s[:, k, mm * 128:(mm + 1) * 128],
                                         rhs=hidT_b[:, k], start=(k == 0),
                                         stop=(k == C - 1))
                for mm in range(C):
                    nc.vector.tensor_scalar_add(out=qsb[:, mm], in0=p_q[:, mm],
                                                scalar1=qbw_s[:, mm:mm + 1])
                if DEBUG:
                    nc.sync.dma_start(out=dbg["q"], in_=qsb[:])
                ths = [emit_tanh(ps, m, 0) for m, ps in enumerate(pss)]
                for m in range(4, C):
                    ths.append(emit_tanh(emit_score_mm(enc_b, m), m, 0))
            else:
                for m in range(C):
                    ths.append(emit_tanh(emit_score_mm(enc_b, m), m, b))
            # score matmul with column-replicated V: every partition of the
            # PSUM bank receives the same score row -> softmax runs wide and
            # the softmaxed weights are already broadcast for the weighted sum
            psc = p_sc.tile([128, T, S], F32, tag="psc")
            for m in range(C):
                nc.tensor.matmul(out=psc[:], lhsT=vw_s[:, m], rhs=ths[m][:],
                                 start=(m == 0), stop=(m == C - 1))
            e = s1small.tile([128, T, S], BF16, tag="e")
            nc.scalar.activation(out=e[:], in_=psc[:], func=AF.Exp)
            rs = s1small.tile([128, T], F32, tag="rs")
            nc.vector.reduce_sum(out=rs[:], in_=e[:], axis=AX.X)
            rc = s1small.tile([128, T], F32, tag="rc")
            nc.vector.reciprocal(out=rc[:], in_=rs[:])
            en = s1small.tile([128, T, S], BF16, tag="en")
            nc.vector.tensor_tensor(out=en[:], in0=e[:], in1=_bcast_last(rc[:], S),
                                    op=OP.mult)
            for c in range(C):
                pr = prodp.tile([128, T, S], BF16, tag="pr")
                encv = enc_b[:, c, :].rearrange("p (t s) -> p t s", s=S)
                nc.vector.tensor_tensor(out=pr[:], in0=encv, in1=en[:],
                                        op=OP.mult)
                nc.vector.reduce_sum(out=ctxT_f[:, c, b, :], in_=pr[:], axis=AX.X)
                if b == BL - 1:
                    # per-chunk bf16 copy: lets stage-2's k=c matmuls start
                    # while later chunks still reduce
                    nc.vector.tensor_copy(out=ctxT_b[:, c], in_=ctxT_f[:, c])
    if DEBUG:
        nc.sync.dma_start(out=dbg["ctxT"], in_=ctxT_f[:])

    # =================== STAGE 2: context GRU ===================
    # xm for all steps: ctx_in @ ctx_kernel (+ all z/r biases, + b0 of h)
    with tc.tile_pool(name="ps_xm", bufs=6, space="PSUM") as ps_xm:
        for g in range(3):
            for half in range(2):
                pxm = ps_xm.tile([128, 4, BL, T], F32, tag="pxm")
                for cc in range(4):
                    c = half * 4 + cc
                    col0 = g * D + c * 128
                    for k in range(C):
                        nc.tensor.matmul(out=pxm[:, cc],
                                         lhsT=ctxk_s[:, k, col0:col0 + 128],
                                         rhs=ctxT_b[:, k], start=(k == 0),
                                         stop=(k == C - 1))
                for cc in range(4):
                    c = half * 4 + cc
                    nc.vector.tensor_scalar_add(
                        out=xg[g][:, c], in0=pxm[:, cc],
                        scalar1=cbx_s[:, g * C + c:g * C + c + 1])

    # decoder weights + utt weights: sync queue, ordered behind the enc
    # stream; slot-release semaphores sequence the actual transfers
    decrk_tiles = []
    for k in range(C):
        dk = streamp.tile([128, G3], BF16, tag="st", name=f"decrk{k}")
        nc.sync.dma_start(out=dk[:], in_=ins["decrk"][k])
        decrk_tiles.append(dk)
    # utt weights before deckA: their slots free at stage-1 end, and q_utt
    # runs inside the GRU gaps
    w1u_s = ld(wsmall, ins["w1u"], [128, C, U], BF16, "wA")
    w2u_s = ld(wsmall, ins["w2u"], [128, C, U], BF16, "wB")
    deckA_s = ld(slot1, ins["deckA"], [128, C, G3], BF16, "slot1", chunked=True)

    h_f = None
    ps_hmd = es.enter_context(tc.tile_pool(name="ps_hmd", bufs=2, space="PSUM"))
    hmd_sb = pers.tile([128, 3, C, BL], F32, tag="hmd_sb")

    with tc.tile_pool(name="ps_hm", bufs=2, space="PSUM") as ps_hm:
        for t in range(T):
            phm = ps_hm.tile([128, 3, C, BL], F32, tag="phm")
            for g in (1, 2, 0):   # r first: it gates the candidate chain
                for c in range(C):
                    col0 = g * D + c * 128
                    if t > 0:
                        # hm = h_{t-1} @ ctx_rec_kernel (h0 == 0: skip matmuls)
                        for k in range(C):
                            nc.tensor.matmul(out=phm[:, g, c],
                                             lhsT=ctxrk_s[:, k, col0:col0 + 128],
                                             rhs=seq_b[:, k, :, t - 1],
                                             start=(k == 0),
                                             stop=(k == C - 1 and g != 2))
                    if g == 2:
                        # += ctx_bias[1] h-part (rank-1 augmentation)
                        nc.tensor.matmul(out=phm[:, g, c],
                                         lhsT=cb1h_s[:, c * 128:(c + 1) * 128],
                                         rhs=ones_s[:], start=(t == 0),
                                         stop=True)
            if t < C:
                # fill the PE gate-chain wait with stage-4 hm_dec chunk t.
                # Each chunk is a self-contained start/stop group (a PSUM
                # accumulation group must not interleave with other groups
                # in flight on other banks: any start=True clears the
                # tracker); chunks accumulate in SBUF via DVE.
                k = t
                dk = decrk_tiles[k]
                phmd_k = ps_hmd.tile([128, 3, C, BL], F32, tag="phmd_k")
                for g in range(3):
                    for c in range(C):
                        col0 = g * D + c * 128
                        nc.tensor.matmul(out=phmd_k[:, g, c],
                                         lhsT=dk[:, col0:col0 + 128],
                                         rhs=hidT_b[:, k], start=True,
                                         stop=(k != C - 1 or g != 2))
                        if k == C - 1 and g == 2:
                            nc.tensor.matmul(out=phmd_k[:, g, c],
                                             lhsT=db1h_s[:, c * 128:(c + 1) * 128],
                                             rhs=ones_s[:], start=False, stop=True)
                if k == 0:
                    nc.vector.tensor_copy(out=hmd_sb[:], in_=phmd_k[:])
                else:
                    nc.vector.tensor_tensor(out=hmd_sb[:], in0=hmd_sb[:],
                                            in1=phmd_k[:], op=OP.add)
            if t == 5:
                pqu_es = ExitStack()
                pqu = pqu_es.enter_context(
                    tc.tile_pool(name="pqu", bufs=1, space="PSUM"))
                p_qu = pqu.tile([128, C, BL], F32)
            if t in (5, 6):
                # utt query, split across two late-GRU PE gap windows
                for m in range((t - 5) * 4, (t - 4) * 4):
                    for k in range(C):
                        nc.tensor.matmul(
                            out=p_qu[:, m],
                            lhsT=w2u_s[:, k, m * 128:(m + 1) * 128],
                            rhs=hidT_b[:, k], start=(k == 0),
                            stop=(k == C - 1))
                    nc.vector.tensor_scalar_add(out=qu_s[:, m], in0=p_qu[:, m],
                                                scalar1=qbu_s[:, m:m + 1])
                if t == 6:
                    pqu_es.close()
            # sigmoid(x) == (tanh(x/2)+1)/2; affine parts are folded into the
            # host-scaled h-kernel columns (x2) and mask (x -0.5), keeping the
            # whole kernel on one ACT function table (Tanh/Exp/Copy).
            mask_bc = _bcast_mid(mask_s[:, t, :], C)
            tz = gtmp.tile([128, C, BL], F32, tag="tz")
            tr = gtmp.tile([128, C, BL], F32, tag="tr")
            if t == 0:
                # h0 == 0: xm is the whole gate input; hm-h is bias only
                nc.scalar.activation(out=tr[:], in_=xg[1][:, :, :, 0],
                                     func=AF.Tanh, scale=0.5)
                nc.scalar.activation(out=tz[:], in_=xg[0][:, :, :, 0],
                                     func=AF.Tanh, scale=0.5)
            else:
                rin = gtmp.tile([128, C, BL], F32, tag="rin")
                nc.vector.tensor_tensor(out=rin[:], in0=xg[1][:, :, :, t],
                                        in1=phm[:, 1], op=OP.add)
                nc.scalar.activation(out=tr[:], in_=rin[:], func=AF.Tanh, scale=0.5)
                zin = gtmp.tile([128, C, BL], F32, tag="zin")
                nc.vector.tensor_tensor(out=zin[:], in0=xg[0][:, :, :, t],
                                        in1=phm[:, 0], op=OP.add)
                nc.scalar.activation(out=tz[:], in_=zin[:], func=AF.Tanh, scale=0.5)
            # rhh2 = (tanh_r + 1) * hh == 2*r*hh; xg-h is host-doubled
            rhh = gtmp.tile([128, C, BL], F32, tag="rhh")
            nc.vector.scalar_tensor_tensor(out=rhh[:], in0=tr[:], scalar=1.0,
                                           in1=phm[:, 2], op0=OP.add, op1=OP.mult)
            cin = gtmp.tile([128, C, BL], F32, tag="cin")
            nc.vector.tensor_tensor(out=cin[:], in0=xg[2][:, :, :, t], in1=rhh[:],
                                    op=OP.add)
            # zcm = (1-z)*mask == (tanh_z - 1) * (-0.5*mask)
            zcm = gtmp.tile([128, C, BL], F32, tag="zcm")
            nc.vector.scalar_tensor_tensor(out=zcm[:], in0=tz[:], scalar=-1.0,
                                           in1=mask_bc, op0=OP.add, op1=OP.mult)
            # h' = h + zcm*(cand - h) = (h - zcm*h) + zcm*cand;
            # (h - zcm*h) is precomputed off the candidate critical path
            h_f2 = hstate.tile([128, C, BL], F32, tag="h_f")
            if t > 0:
                hz1 = gtmp.tile([128, C, BL], F32, tag="hz1")
                nc.vector.tensor_tensor(out=hz1[:], in0=h_f[:], in1=zcm[:],
                                        op=OP.mult)
                hm1 = gtmp.tile([128, C, BL], F32, tag="hm1")
                nc.vector.tensor_tensor(out=hm1[:], in0=h_f[:], in1=hz1[:],
                                        op=OP.subtract)
            cand = gtmp.tile([128, C, BL], F32, tag="cand")
            nc.scalar.activation(out=cand[:], in_=cin[:], func=AF.Tanh, scale=0.5)
            if t == 0:
                nc.vector.tensor_tensor(out=seq_b[:, :, :, 0], in0=cand[:],
                                        in1=zcm[:], op=OP.mult)
                nc.vector.tensor_tensor(out=h_f2[:], in0=cand[:], in1=zcm[:],
                                        op=OP.mult)
            else:
                t2 = gtmp.tile([128, C, BL], F32, tag="t2")
                nc.vector.tensor_tensor(out=t2[:], in0=cand[:], in1=zcm[:],
                                        op=OP.mult)
                nc.vector.tensor_tensor(out=seq_b[:, :, :, t], in0=hm1[:],
                                        in1=t2[:], op=OP.add)
                nc.vector.tensor_tensor(out=h_f2[:], in0=hm1[:], in1=t2[:],
                                        op=OP.add)
            h_f = h_f2
    if DEBUG:
        dbg_seq_f = pers.tile([128, C, BL, T], F32, tag="dbg_seq_f")
        nc.vector.tensor_copy(out=dbg_seq_f[:], in_=seq_b[:])
        nc.sync.dma_start(out=dbg["seq"], in_=dbg_seq_f[:])

    # =================== STAGE 3: utterance attention ===================
    # rotate slots: deckB into ctxrk's freed slot (sync queue)
    deckB_s = ld(slot2, ins["deckB"], [128, C, G3], BF16, "slot2", chunked=True)

    with tc.tile_pool(name="ps_su", bufs=4, space="PSUM") as ps_su, \
         tc.tile_pool(name="ps_scu", bufs=1, space="PSUM") as ps_scu, \
         tc.tile_pool(name="s3tmp", bufs=3) as s3tmp:
        sus = []
        for m in range(C):
            psu = ps_su.tile([128, BL, T], F32, tag="psu")
            for k in range(C):
                nc.tensor.matmul(out=psu[:], lhsT=w1u_s[:, k, m * 128:(m + 1) * 128],
                                 rhs=seq_b[:, k], start=(k == 0), stop=(k == C - 1))
            qn = s3tmp.tile([128, BL, T], F32, tag="qn")
            nc.vector.tensor_tensor(out=qn[:], in0=psu[:],
                                    in1=_bcast_last(qu_s[:, m, :], T), op=OP.add)
            su = su_pool.tile([128, BL, T], BF16, tag="su")
            nc.scalar.activation(out=su[:], in_=qn[:], func=AF.Tanh)
            sus.append(su)
        pscu = ps_scu.tile([128, BL, T], F32)
        for m in range(C):
            nc.tensor.matmul(out=pscu[:], lhsT=vu_s[:, m], rhs=sus[m][:],
                             start=(m == 0), stop=(m == C - 1))
        eu = s3tmp.tile([128, BL, T], BF16, tag="eu")
        nc.scalar.activation(out=eu[:], in_=pscu[:], func=AF.Exp)
        rsu = s3tmp.tile([128, BL], F32, tag="rsu")
        nc.vector.reduce_sum(out=rsu[:], in_=eu[:], axis=AX.X)
        rcu = s3tmp.tile([128, BL], F32, tag="rcu")
        nc.vector.reciprocal(out=rcu[:], in_=rsu[:])
        eun = s3tmp.tile([128, BL, T], BF16, tag="eun")
        nc.vector.tensor_tensor(out=eun[:], in0=eu[:], in1=_bcast_last(rcu[:], T),
                                op=OP.mult)
        for c in range(C):
            pru = s3tmp.tile([128, BL, T], BF16, tag="pru")
            nc.vector.tensor_tensor(out=pru[:], in0=seq_b[:, c], in1=eun[:],
                                    op=OP.mult)
            nc.vector.reduce_sum(out=ctxvT_f[:, c, :], in_=pru[:], axis=AX.X)
            nc.vector.tensor_copy(out=ctxvT_b[:, c], in_=ctxvT_f[:, c])
    if DEBUG:
        nc.sync.dma_start(out=dbg["ctxv"], in_=ctxvT_f[:])

    # =================== STAGE 4: decoder GRU step ===================
    # (hm_dec was computed during stage 2; biases already augmented)
    with tc.tile_pool(name="ps_xmd", bufs=1, space="PSUM") as ps_xmd, \
         tc.tile_pool(name="ps_out", bufs=2, space="PSUM") as ps_out:
        pxmd = ps_xmd.tile([128, 3, C, BL], F32)
        for g in range(3):
            for c in range(C):
                col0 = g * D + c * 128
                for k in range(KD):
                    if k < C:
                        lhsT = deckA_s[:, k, col0:col0 + 128]
                        rhs = ctxvT_b[:, k]
                    else:
                        lhsT = deckB_s[:, k - C, col0:col0 + 128]
                        rhs = xembT_b[:, k - C]
                    nc.tensor.matmul(out=pxmd[:, g, c], lhsT=lhsT, rhs=rhs,
                                     start=(k == 0), stop=(k == KD - 1))
        # copy to SBUF with dec biases folded (z,r: b0+b1; h: 2*b0 host-scaled)
        xmd_sb = s4tmp.tile([128, 3, C, BL], F32, tag="xmd_sb")
        for g in range(3):
            for c in range(C):
                nc.vector.tensor_scalar_add(out=xmd_sb[:, g, c], in0=pxmd[:, g, c],
                                            scalar1=dbx_s[:, g * C + c:g * C + c + 1])

        tz = s4tmp.tile([128, C, BL], F32, tag="tz4")
        tr = s4tmp.tile([128, C, BL], F32, tag="tr4")
        rin = s4tmp.tile([128, C, BL], F32, tag="rin4")
        nc.vector.tensor_tensor(out=rin[:], in0=xmd_sb[:, 1], in1=hmd_sb[:, 1],
                                op=OP.add)
        nc.scalar.activation(out=tr[:], in_=rin[:], func=AF.Tanh, scale=0.5)
        zin = s4tmp.tile([128, C, BL], F32, tag="zin4")
        nc.vector.tensor_tensor(out=zin[:], in0=xmd_sb[:, 0], in1=hmd_sb[:, 0],
                                op=OP.add)
        nc.scalar.activation(out=tz[:], in_=zin[:], func=AF.Tanh, scale=0.5)
        rhh = s4tmp.tile([128, C, BL], F32, tag="rhh4")
        nc.vector.scalar_tensor_tensor(out=rhh[:], in0=tr[:], scalar=1.0,
                                       in1=hmd_sb[:, 2], op0=OP.add, op1=OP.mult)
        cin = s4tmp.tile([128, C, BL], F32, tag="cin4")
        nc.vector.tensor_tensor(out=cin[:], in0=xmd_sb[:, 2], in1=rhh[:], op=OP.add)
        cand = s4tmp.tile([128, C, BL], F32, tag="cand4")
        nc.scalar.activation(out=cand[:], in_=cin[:], func=AF.Tanh, scale=0.5)
        # zcm = (1-z) == (tanh_z - 1) * (-0.5)
        zcm = s4tmp.tile([128, C, BL], F32, tag="zcm4")
        nc.vector.tensor_scalar(out=zcm[:], in0=tz[:], scalar1=-1.0, scalar2=-0.5,
                                op0=OP.add, op1=OP.mult)
        d1 = s4tmp.tile([128, C, BL], F32, tag="d14")
        nc.vector.tensor_tensor(out=d1[:], in0=cand[:], in1=hidT_f[:], op=OP.subtract)
        d2 = s4tmp.tile([128, C, BL], F32, tag="d24")
        nc.vector.tensor_tensor(out=d2[:], in0=d1[:], in1=zcm[:], op=OP.mult)
        stT = s4tmp.tile([128, C, BL], F32, tag="stT")
        nc.vector.tensor_tensor(out=stT[:], in0=hidT_f[:], in1=d2[:], op=OP.add)
        if DEBUG:
            nc.sync.dma_start(out=dbg["xmd"], in_=xmd_sb[:])
            nc.sync.dma_start(out=dbg["hmd"], in_=hmd_sb[:])
            nc.sync.dma_start(out=dbg["stT"], in_=stT[:])

        out_sb = s4tmp.tile([BL, D], F32, tag="out_sb")
        for c in range(C):
            po = ps_out.tile([BL, 128], F32, tag="po")
            nc.tensor.transpose(out=po[:], in_=stT[:, c], identity=ident_s[:])
            nc.vector.tensor_copy(out=out_sb[:, c * 128:(c + 1) * 128], in_=po[:])
        nc.sync.dma_start(out=ins["out"], in_=out_sb[:])

    es.close()


# ---------------------------------------------------------------------------
# Host side
# ---------------------------------------------------------------------------

_NC_CACHE = {}


def _get_nc():
    key = ("prog", DEBUG)
    if key not in _NC_CACHE:
        _NC_CACHE[key] = build()
    return _NC_CACHE[key]


def _bf(a):
    return np.ascontiguousarray(a.astype(ml_dtypes.bfloat16))


def _f32(a):
    return np.ascontiguousarray(a.astype(np.float32))


def _chunked_T(w):
    """[D_in, N] -> [128, D_in//128, N]: row-chunked for per-k lhsT tiles."""
    d_in, n = w.shape
    return np.ascontiguousarray(w.reshape(d_in // 128, 128, n).transpose(1, 0, 2))


def prepare_in_maps(inputs):
    x = np.asarray(inputs["x"]).astype(np.int64).reshape(B)
    hidden = _f32(np.asarray(inputs["hidden"]))            # [64, 1024]
    enc = _f32(np.asarray(inputs["encoder_outputs"]))      # [64, 10, 50, 1024]
    maskf = np.asarray(inputs["context_mask"]).astype(np.float32)  # [64, 10]
    emb = np.asarray(inputs["embed_table"])                # [V, 1024]

    x_emb = emb[x].astype(np.float32)                       # [64, 1024]

    def tmajor(a2d):  # [B, D] -> [128, C, B]
        return np.ascontiguousarray(
            a2d.T.reshape(C, 128, a2d.shape[0]).transpose(1, 0, 2))

    def dbl_h(w):
        # double the candidate-gate columns: folds the sigmoid-via-tanh
        # affine rescale into the input kernels (see _emit)
        w = np.array(w, np.float32, copy=True)
        w[:, 2 * D:] *= 2.0
        return w

    w1w = _bf(_chunked_T(np.asarray(inputs["w1_word"], np.float32)))
    w2w = _bf(_chunked_T(np.asarray(inputs["w2_word"], np.float32)))
    w1u = _bf(_chunked_T(np.asarray(inputs["w1_utt"], np.float32)))
    w2u = _bf(_chunked_T(np.asarray(inputs["w2_utt"], np.float32)))
    ctxk = _bf(_chunked_T(dbl_h(np.asarray(inputs["ctx_kernel"], np.float32))))
    ctxrk = _bf(_chunked_T(np.asarray(inputs["ctx_rec_kernel"], np.float32)))
    deck_full = dbl_h(np.asarray(inputs["dec_kernel"], np.float32))  # [2048, 3072]
    deckA = _bf(_chunked_T(deck_full[:D]))
    deckB = _bf(_chunked_T(deck_full[D:]))
    decrk = _bf(np.asarray(inputs["dec_rec_kernel"], np.float32)
                .reshape(C, 128, G3))                             # chunk-major

    def vrep(v):   # [U, 1] -> [128, C, 128] (chunked, bcast across columns)
        vc = np.asarray(v, np.float32).reshape(C, 128).T     # [128, C]
        return _bf(np.broadcast_to(vc[:, :, None], (128, C, 128)))

    vw = vrep(inputs["v_word"])
    vu = vrep(inputs["v_utt"])

    def mchunk(v):   # [U] -> [128, C]
        return _f32(np.asarray(v, np.float32).reshape(C, 128).T)

    qb_w = mchunk(np.asarray(inputs["b1_word"], np.float32)
                  + np.asarray(inputs["b2_word"], np.float32))
    qb_u = mchunk(np.asarray(inputs["b1_utt"], np.float32)
                  + np.asarray(inputs["b2_utt"], np.float32))

    cbias = np.asarray(inputs["ctx_bias"], np.float32)      # [2, 3072]
    dbias = np.asarray(inputs["dec_bias"], np.float32)      # [2, 3072]

    def gate_bias(bias2):
        return np.concatenate([
            bias2[0, :D] + bias2[1, :D],
            bias2[0, D:2 * D] + bias2[1, D:2 * D],
            2.0 * bias2[0, 2 * D:],      # candidate gate host-doubled
        ])

    cbx = _f32(gate_bias(cbias).reshape(3 * C, 128).T)       # [128, 24]
    dbx = _f32(gate_bias(dbias).reshape(3 * C, 128).T)
    cb1h = _bf(cbias[1, 2 * D:].reshape(1, D))
    db1h = _bf(dbias[1, 2 * D:].reshape(1, D))

    ones_b = _bf(np.ones((1, BL), np.float32))
    ident = _f32(np.eye(128, dtype=np.float32))

    enc_r = enc.reshape(B, R, D)

    in_maps = []
    for core in range(NCORES):
        sl = slice(core * BL, (core + 1) * BL)
        # [8, 500, 1024] -> [b][p, c, r]
        enc_t = np.ascontiguousarray(
            enc_r[sl].transpose(0, 2, 1)                     # [8, 1024, 500]
            .reshape(BL, C, 128, R)
            .transpose(0, 2, 1, 3))                          # [8, 128, C, 500]
        hid_c = hidden[sl]
        # pre-scaled by -0.5 for the (tanh_z - 1) * (-mask/2) update form
        mask_t = np.ascontiguousarray(
            np.broadcast_to(-0.5 * maskf[sl].T[None, :, :], (128, T, BL)))
        in_maps.append({
            "enc_t": _bf(enc_t),
            "hidT_f": _f32(tmajor(hid_c)),
            "hidT_b": _bf(tmajor(hid_c)),
            "xembT_b": _bf(tmajor(x_emb[sl])),
            "w1w": w1w, "w2w": w2w, "vw_rep": vw,
            "w1u": w1u, "w2u": w2u, "vu_rep": vu,
            "ctxk": ctxk, "ctxrk": ctxrk,
            "deckA": deckA, "deckB": deckB, "decrk": decrk,
            "qb_w": qb_w, "qb_u": qb_u, "cbx": cbx,
            "cb1h_b": cb1h, "dbx": dbx, "db1h_b": db1h,
            "mask_t": _f32(mask_t),
            "ones_b": ones_b, "ident": ident,
        })
    return in_maps


def run(inputs):
    nc = _get_nc()
    in_maps = prepare_in_maps(inputs)
    res = run_bass_kernel_spmd(nc, in_maps, list(range(NCORES)))
    out = np.concatenate([res.results[c]["out"] for c in range(NCORES)], axis=0)
    return np.ascontiguousarray(out.astype(np.float32)), res


def kernel(**inputs):
    out, _ = run(inputs)
    return out, out



# revision 8
# speedup vs baseline: 1.5827x; 1.5827x over previous
"""Trainium2 Bass kernel for nn_Decoder_55688545960558 (v2, fp8).

Hierarchical-attention GRU decoder step, data-parallel over batch
(64 -> 8 per core), no collectives.

Key differences vs v1:
- All attention weights, enc, and the context-GRU weights in fp8e4
  (validated: rel_err ~2e-3); decoder GRU's recurrent kernel stays bf16.
  fp8 matmuls use DoubleRow perf mode (2 k-chunks per instruction).
- Word-attention softmax: exp is NOT normalized; the weighted sums are
  scaled by 1/Z at the end (saves a [128,500] DVE pass per batch).
- The weighted-sum multiplies are split DVE/GpSimd; reduce on DVE.
- All gate biases enter PSUM via rank-1 ones-matmuls; PSUM->SBUF moves
  are single wide ops (no per-chunk scalar adds).
- Output stays feature-major [128, C, BL]; the host transposes.
"""

from contextlib import ExitStack

import numpy as np
import ml_dtypes

import concourse.bass as bass
import concourse.mybir as mybir
import concourse.tile as tile
from concourse import bacc
from concourse.bass_utils import run_bass_kernel_spmd

F32 = mybir.dt.float32
BF16 = mybir.dt.bfloat16
FP8 = mybir.dt.float8e4
AF = mybir.ActivationFunctionType
OP = mybir.AluOpType
AX = mybir.AxisListType
DR = mybir.MatmulPerfMode.DoubleRow

NCORES = 8
B = 64
BL = B // NCORES  # 8
T = 10
S = 50
R = T * S         # 500
D = 1024
U = 1024
C = D // 128      # 8
CP = C // 2       # 4 k-pairs for DoubleRow
G3 = 3 * D        # 3072

DEBUG = False


def _bcast_mid(ap, n):
    """Insert a 0-stride broadcast dim of size n as dim 1 (after partitions)."""
    return bass.AP(tensor=ap.tensor, offset=ap.offset,
                   ap=[ap.ap[0], [0, n]] + list(ap.ap[1:]))


def _bcast_last(ap, n):
    return bass.AP(tensor=ap.tensor, offset=ap.offset,
                   ap=list(ap.ap) + [[0, n]])


def build():
    nc = bacc.Bacc("TRN2", target_bir_lowering=False, debug=False,
                   num_devices=NCORES)

    def din(name, shape, dt):
        return nc.dram_tensor(name, list(shape), dt, kind="ExternalInput").ap()

    ins = {}
    ins["enc"] = din("enc_t", [BL, 128, C, R], FP8)
    ins["hidT_f"] = din("hidT_f", [128, C, BL], F32)
    ins["hidT_b"] = din("hidT_b", [128, C, BL], BF16)
    ins["hidT_8"] = din("hidT_8", [128, C, BL], FP8)
    ins["xembT_8"] = din("xembT_8", [128, C, BL], FP8)
    ins["w1w"] = din("w1w", [128, C, U], FP8)
    ins["w2w"] = din("w2w", [128, C, U], FP8)
    ins["vw"] = din("vw_rep", [128, C, 128], FP8)
    ins["w1u"] = din("w1u", [128, C, U], FP8)
    ins["w2u"] = din("w2u", [128, C, U], FP8)
    ins["vu"] = din("vu_rep", [128, C, 128], FP8)
    ins["ctxk"] = din("ctxk", [128, C, G3], FP8)
    ins["ctxrk"] = din("ctxrk", [128, C, G3], FP8)
    ins["deckA"] = din("deckA", [128, C, G3], FP8)
    ins["deckB"] = din("deckB", [128, C, G3], FP8)
    ins["decrk"] = din("decrk", [C, 128, G3], BF16)
    ins["qb_w"] = din("qb_w", [128, C], F32)
    ins["qb_u"] = din("qb_u", [128, C], F32)
    ins["cbx_row"] = din("cbx_row", [1, G3], BF16)
    ins["cb1h"] = din("cb1h_b", [1, D], BF16)
    ins["dbx_row"] = din("dbx_row", [1, G3], BF16)
    ins["db1h"] = din("db1h_b", [1, D], BF16)
    ins["mask"] = din("mask_t", [128, T, BL], F32)   # pre-scaled by -0.5
    ins["ones"] = din("ones_b", [1, BL * T], BF16)

    ins["out"] = nc.dram_tensor("out", [128, C, BL], F32,
                                kind="ExternalOutput").ap()
    dbg = {}
    if DEBUG:
        dbg["q"] = nc.dram_tensor("dbg_q", [128, C, BL], F32,
                                  kind="ExternalOutput").ap()
        dbg["ctx"] = nc.dram_tensor("dbg_ctx", [128, C, BL, T], F32,
                                    kind="ExternalOutput").ap()
        dbg["seq"] = nc.dram_tensor("dbg_seq", [128, C, BL, T], F32,
                                    kind="ExternalOutput").ap()
        dbg["ctxv"] = nc.dram_tensor("dbg_ctxv", [128, C, BL], F32,
                                     kind="ExternalOutput").ap()
        dbg["hmd"] = nc.dram_tensor("dbg_hmd", [128, 3, C, BL], F32,
                                    kind="ExternalOutput").ap()
        dbg["xmd"] = nc.dram_tensor("dbg_xmd", [128, 3, C, BL], F32,
                                    kind="ExternalOutput").ap()
    ins["dbg"] = dbg

    with nc.allow_low_precision(reason="bf16/fp8 activations by design"):
        with tile.TileContext(nc) as tc:
            _emit(nc, tc, ins)
    nc.compile()
    return nc


def _emit(nc, tc, ins):
    dbg = ins["dbg"]
    es = ExitStack()

    pers = es.enter_context(tc.tile_pool(name="pers", bufs=1))
    wsA = es.enter_context(tc.tile_pool(name="wsA", bufs=1))    # w1w -> w1u
    wsB = es.enter_context(tc.tile_pool(name="wsB", bufs=1))    # w2w -> w2u
    gruw = es.enter_context(tc.tile_pool(name="gruw", bufs=1))  # ctxk/ctxrk
    decw = es.enter_context(tc.tile_pool(name="decw", bufs=1))  # deckA/deckB
    encp = es.enter_context(tc.tile_pool(name="encp", bufs=3))
    drkp = es.enter_context(tc.tile_pool(name="drkp", bufs=2))
    thp = es.enter_context(tc.tile_pool(name="thp", bufs=2))
    ep = es.enter_context(tc.tile_pool(name="ep", bufs=2))
    prp = es.enter_context(tc.tile_pool(name="prp", bufs=2))
    s1small = es.enter_context(tc.tile_pool(name="s1small", bufs=2))
    gtmp = es.enter_context(tc.tile_pool(name="gtmp", bufs=3))
    hstate = es.enter_context(tc.tile_pool(name="hstate", bufs=2))
    s34 = es.enter_context(tc.tile_pool(name="s34", bufs=2))

    def ld(pool, dram_ap, shape, dt, name, chunked=False):
        t = pool.tile(list(shape), dt, tag=name, name=name)
        if chunked:
            for k in range(shape[1]):
                nc.sync.dma_start(out=t[:, k], in_=dram_ap[:, k])
        else:
            nc.sync.dma_start(out=t[:], in_=dram_ap)
        return t

    # ---------------- DMA: critical-path order on the sync queue ----------
    hidT_8 = ld(pers, ins["hidT_8"], [128, C, BL], FP8, "hidT_8")
    qbw_s = ld(pers, ins["qb_w"], [128, C], F32, "qbw")
    vw_s = ld(pers, ins["vw"], [128, C, 128], FP8, "vw")
    w1w_s = wsA.tile([128, C, U], FP8, tag="wA", name="w1w")
    enc_b0 = encp.tile([128, C, R], FP8, tag="enc", name="enc0")
    for k in range(C):
        nc.sync.dma_start(out=w1w_s[:, k], in_=ins["w1w"][:, k])
        nc.sync.dma_start(out=enc_b0[:, k], in_=ins["enc"][0][:, k])
    w2w_s = ld(wsB, ins["w2w"], [128, C, U], FP8, "wB")
    enc_tiles = [enc_b0]
    for b in range(1, BL):
        enc_tiles.append(ld(encp, ins["enc"][b], [128, C, R], FP8, "enc",
                            chunked=True))
    hidT_f = ld(pers, ins["hidT_f"], [128, C, BL], F32, "hidT_f")
    hidT_b = ld(pers, ins["hidT_b"], [128, C, BL], BF16, "hidT_b")
    xembT_8 = ld(pers, ins["xembT_8"], [128, C, BL], FP8, "xembT_8")
    qbu_s = ld(pers, ins["qb_u"], [128, C], F32, "qbu")
    mask_s = ld(pers, ins["mask"], [128, T, BL], F32, "mask")
    ones_s = ld(pers, ins["ones"], [1, BL * T], BF16, "ones")
    cbx_s = ld(pers, ins["cbx_row"], [1, G3], BF16, "cbx")
    cb1h_s = ld(pers, ins["cb1h"], [1, D], BF16, "cb1h")
    dbx_s = ld(pers, ins["dbx_row"], [1, G3], BF16, "dbx")
    db1h_s = ld(pers, ins["db1h"], [1, D], BF16, "db1h")
    ctxk_s = ld(gruw, ins["ctxk"], [128, C, G3], FP8, "ctxk", chunked=True)
    ctxrk_s = ld(gruw, ins["ctxrk"], [128, C, G3], FP8, "ctxrk", chunked=True)
    w1u_s = ld(wsA, ins["w1u"], [128, C, U], FP8, "wA")
    w2u_s = ld(wsB, ins["w2u"], [128, C, U], FP8, "wB")
    vu_s = ld(pers, ins["vu"], [128, C, 128], FP8, "vu")
    decrk_tiles = []
    for k in range(C):
        dk = drkp.tile([128, G3], BF16, tag="drk", name=f"decrk{k}")
        nc.sync.dma_start(out=dk[:], in_=ins["decrk"][k])
        decrk_tiles.append(dk)
    deckB_s = ld(decw, ins["deckB"], [128, C, G3], FP8, "deckB", chunked=True)
    deckA_s = ld(decw, ins["deckA"], [128, C, G3], FP8, "deckA", chunked=True)

    # cross-stage activations
    qsb = pers.tile([128, C, BL], F32, tag="qsb")
    qu_s = pers.tile([128, C, BL], F32, tag="qu")
    ctx8 = pers.tile([128, C, BL, T], FP8, tag="ctx8")
    seq8 = pers.tile([128, C, BL, T], FP8, tag="seq8")
    xg = [pers.tile([128, C, BL, T], BF16, tag=f"xg{g}", name=f"xg{g}")
          for g in range(3)]
    hmd_sb = pers.tile([128, 3, C, BL], F32, tag="hmd_sb")
    bh_sb = pers.tile([128, 3, C, BL], F32, tag="bh_sb")
    ctxv8 = pers.tile([128, C, BL], FP8, tag="ctxv8")

    # =================== STAGE 1: word attention ===================
    with tc.tile_pool(name="ps_score", bufs=5, space="PSUM") as p_score, \
         tc.tile_pool(name="pq", bufs=1, space="PSUM") as pq:
        def score_group(enc_b, m):
            ps = p_score.tile([128, R], F32, tag="ps")
            for kp in range(CP):
                nc.tensor.matmul(out=ps[:],
                                 lhsT=w1w_s[:, 2 * kp:2 * kp + 2,
                                            m * 128:(m + 1) * 128],
                                 rhs=enc_b[:, 2 * kp:2 * kp + 2],
                                 start=(kp == 0), stop=(kp == CP - 1),
                                 perf_mode=DR)
            return ps

        def q_matmuls(w_s, qb, out_sb):
            p_q = pq.tile([128, C, BL], F32, tag="pq")
            for mm in range(C):
                for kp in range(CP):
                    nc.tensor.matmul(out=p_q[:, mm],
                                     lhsT=w_s[:, 2 * kp:2 * kp + 2,
                                              mm * 128:(mm + 1) * 128],
                                     rhs=hidT_8[:, 2 * kp:2 * kp + 2],
                                     start=(kp == 0), stop=(kp == CP - 1),
                                     perf_mode=DR)
            for mm in range(C):
                nc.vector.tensor_scalar_add(out=out_sb[:, mm], in0=p_q[:, mm],
                                            scalar1=qb[:, mm:mm + 1])

        for b in range(BL):
            enc_b = enc_tiles[b]
            th = thp.tile([128, C, R], FP8, tag="th")
            if b == 0:
                pss = [score_group(enc_b, m) for m in range(4)]
                q_matmuls(w2w_s, qbw_s, qsb)
                if DEBUG:
                    nc.sync.dma_start(out=dbg["q"], in_=qsb[:])
                for m in range(4):
                    nc.scalar.activation(out=th[:, m], in_=pss[m][:],
                                         func=AF.Tanh,
                                         bias=qsb[:, m, b:b + 1])
                for m in range(4, C):
                    ps = score_group(enc_b, m)
                    nc.scalar.activation(out=th[:, m], in_=ps[:],
                                         func=AF.Tanh,
                                         bias=qsb[:, m, b:b + 1])
            else:
                for m in range(C):
                    ps = score_group(enc_b, m)
                    nc.scalar.activation(out=th[:, m], in_=ps[:],
                                         func=AF.Tanh,
                                         bias=qsb[:, m, b:b + 1])
            if b == 0:
                # utt query in the b0 PE shadow (weights already resident)
                q_matmuls(w2u_s, qbu_s, qu_s)
            # V matmul (replicated scores on all partitions)
            psc = p_score.tile([128, R], F32, tag="ps")
            for cp in range(CP):
                nc.tensor.matmul(out=psc[:], lhsT=vw_s[:, 2 * cp:2 * cp + 2],
                                 rhs=th[:, 2 * cp:2 * cp + 2],
                                 start=(cp == 0), stop=(cp == CP - 1),
                                 perf_mode=DR)
            e = s1small.tile([128, T, S], BF16, tag="e")
            nc.scalar.activation(
                out=e[:], in_=psc[:].rearrange("p (t s) -> p t s", s=S),
                func=AF.Exp)
            rs = s1small.tile([128, T], F32, tag="rs")
            nc.vector.reduce_sum(out=rs[:], in_=e[:], axis=AX.X)
            rc = s1small.tile([128, T], F32, tag="rc")
            nc.vector.reciprocal(out=rc[:], in_=rs[:])
            # unnormalized weighted sum: pr = enc * e  (DVE 2 chunks, Pool 6)
            pr = prp.tile([128, C, T, S], BF16, tag="pr")
            encv = enc_b[:].rearrange("p c (t s) -> p c t s", s=S)
            nc.vector.tensor_tensor(out=pr[:, 0:2], in0=encv[:, 0:2],
                                    in1=_bcast_mid(e[:], 2), op=OP.mult)
            nc.gpsimd.tensor_tensor(out=pr[:, 2:8], in0=encv[:, 2:8],
                                    in1=_bcast_mid(e[:], 6), op=OP.mult)
            red = s1small.tile([128, C, T], F32, tag="red")
            nc.vector.reduce_sum(out=red[:], in_=pr[:], axis=AX.X)
            # normalize by 1/Z while converting to fp8
            nc.vector.tensor_tensor(out=ctx8[:, :, b, :], in0=red[:],
                                    in1=_bcast_mid(rc[:], C), op=OP.mult)
    if DEBUG:
        dbg_ctx = pers.tile([128, C, BL, T], F32, tag="dbg_ctx")
        nc.vector.tensor_copy(out=dbg_ctx[:], in_=ctx8[:])
        nc.sync.dma_start(out=dbg["ctx"], in_=dbg_ctx[:])

    # =================== STAGE 2: context GRU ===================
    ctx8v = ctx8[:].rearrange("p c b t -> p c (b t)")
    with tc.tile_pool(name="ps_xm", bufs=3, space="PSUM") as ps_xm:
        for g in range(3):
            for half in range(2):
                pxm = ps_xm.tile([128, 4, BL, T], F32, tag="pxm")
                for cc in range(4):
                    c = half * 4 + cc
                    col0 = g * D + c * 128
                    for kp in range(CP):
                        nc.tensor.matmul(
                            out=pxm[:, cc],
                            lhsT=ctxk_s[:, 2 * kp:2 * kp + 2, col0:col0 + 128],
                            rhs=ctx8v[:, 2 * kp:2 * kp + 2],
                            start=(kp == 0), stop=False, perf_mode=DR)
                    # bias as rank-1 ones-matmul closes the group
                    nc.tensor.matmul(out=pxm[:, cc],
                                     lhsT=cbx_s[:, col0:col0 + 128],
                                     rhs=ones_s[:], start=False, stop=True)
                nc.vector.tensor_copy(out=xg[g][:, half * 4:half * 4 + 4],
                                      in_=pxm[:])

    h_f = None
    ps_hmd = es.enter_context(tc.tile_pool(name="ps_hmd", bufs=2, space="PSUM"))

    with tc.tile_pool(name="ps_hm", bufs=2, space="PSUM") as ps_hm:
        for t in range(T):
            phm = ps_hm.tile([128, 3, C, BL], F32, tag="phm")
            for g in (1, 2, 0):   # r first: it gates the candidate chain
                for c in range(C):
                    col0 = g * D + c * 128
                    if t > 0:
                        for kp in range(CP):
                            nc.tensor.matmul(
                                out=phm[:, g, c],
                                lhsT=ctxrk_s[:, 2 * kp:2 * kp + 2,
                                             col0:col0 + 128],
                                rhs=seq8[:, 2 * kp:2 * kp + 2, :, t - 1],
                                start=(kp == 0),
                                stop=(kp == CP - 1 and g != 2),
                                perf_mode=DR)
                    if g == 2:
                        nc.tensor.matmul(out=phm[:, g, c],
                                         lhsT=cb1h_s[:, c * 128:(c + 1) * 128],
                                         rhs=ones_s[:, :BL], start=(t == 0),
                                         stop=True)
            if 2 <= t < 2 + C:
                # stage-4 hm_dec chunk (decrk streams in during the scan)
                k = t - 2
                dk = decrk_tiles[k]
                phmd_k = ps_hmd.tile([128, 3, C, BL], F32, tag="phmd_k")
                for g in range(3):
                    for c in range(C):
                        col0 = g * D + c * 128
                        nc.tensor.matmul(out=phmd_k[:, g, c],
                                         lhsT=dk[:, col0:col0 + 128],
                                         rhs=hidT_b[:, k], start=True,
                                         stop=(k != C - 1 or g != 2))
                        if k == C - 1 and g == 2:
                            nc.tensor.matmul(
                                out=phmd_k[:, g, c],
                                lhsT=db1h_s[:, c * 128:(c + 1) * 128],
                                rhs=ones_s[:, :BL], start=False, stop=True)
                if k == 0:
                    nc.vector.tensor_copy(out=hmd_sb[:], in_=phmd_k[:])
                else:
                    nc.vector.tensor_tensor(out=hmd_sb[:], in0=hmd_sb[:],
                                            in1=phmd_k[:], op=OP.add)
            # sigmoid(x) == (tanh(x/2)+1)/2; affine parts folded on host
            mask_bc = _bcast_mid(mask_s[:, t, :], C)
            tz = gtmp.tile([128, C, BL], F32, tag="tz")
            tr = gtmp.tile([128, C, BL], F32, tag="tr")
            if t == 0:
                nc.scalar.activation(out=tr[:], in_=xg[1][:, :, :, 0],
                                     func=AF.Tanh, scale=0.5)
                nc.scalar.activation(out=tz[:], in_=xg[0][:, :, :, 0],
                                     func=AF.Tanh, scale=0.5)
            else:
                rin = gtmp.tile([128, C, BL], F32, tag="rin")
                nc.vector.tensor_tensor(out=rin[:], in0=xg[1][:, :, :, t],
                                        in1=phm[:, 1], op=OP.add)
                nc.scalar.activation(out=tr[:], in_=rin[:], func=AF.Tanh,
                                     scale=0.5)
                zin = gtmp.tile([128, C, BL], F32, tag="zin")
                nc.vector.tensor_tensor(out=zin[:], in0=xg[0][:, :, :, t],
                                        in1=phm[:, 0], op=OP.add)
                nc.scalar.activation(out=tz[:], in_=zin[:], func=AF.Tanh,
                                     scale=0.5)
            # rhh = (tanh_r + 1) * hh == 2*r*hh; xg-h host-doubled
            # rhh = (tanh_r + 1) * hh; at t==0 phm[2] is the h-bias only
            rhh = gtmp.tile([128, C, BL], F32, tag="rhh")
            nc.vector.scalar_tensor_tensor(out=rhh[:], in0=tr[:],
                                           scalar=1.0, in1=phm[:, 2],
                                           op0=OP.add, op1=OP.mult)
            cin = gtmp.tile([128, C, BL], F32, tag="cin")
            nc.vector.tensor_tensor(out=cin[:], in0=xg[2][:, :, :, t],
                                    in1=rhh[:], op=OP.add)
            # zcm = (1-z)*mask == (tanh_z - 1) * (-0.5*mask)
            zcm = gtmp.tile([128, C, BL], F32, tag="zcm")
            nc.vector.scalar_tensor_tensor(out=zcm[:], in0=tz[:], scalar=-1.0,
                                           in1=mask_bc, op0=OP.add,
                                           op1=OP.mult)
            h_f2 = hstate.tile([128, C, BL], F32, tag="h_f")
            if t > 0:
                hz1 = gtmp.tile([128, C, BL], F32, tag="hz1")
                nc.vector.tensor_tensor(out=hz1[:], in0=h_f[:], in1=zcm[:],
                                        op=OP.mult)
                hm1 = gtmp.tile([128, C, BL], F32, tag="hm1")
                nc.vector.tensor_tensor(out=hm1[:], in0=h_f[:], in1=hz1[:],
                                        op=OP.subtract)
            cand = gtmp.tile([128, C, BL], F32, tag="cand")
            nc.scalar.activation(out=cand[:], in_=cin[:], func=AF.Tanh,
                                 scale=0.5)
            if t == 0:
                nc.vector.tensor_tensor(out=seq8[:, :, :, 0], in0=cand[:],
                                        in1=zcm[:], op=OP.mult)
                nc.vector.tensor_tensor(out=h_f2[:], in0=cand[:], in1=zcm[:],
                                        op=OP.mult)
            else:
                t2 = gtmp.tile([128, C, BL], F32, tag="t2")
                nc.vector.tensor_tensor(out=t2[:], in0=cand[:], in1=zcm[:],
                                        op=OP.mult)
                nc.vector.tensor_tensor(out=seq8[:, :, :, t], in0=hm1[:],
                                        in1=t2[:], op=OP.add)
                nc.vector.tensor_tensor(out=h_f2[:], in0=hm1[:], in1=t2[:],
                                        op=OP.add)
            h_f = h_f2
    if DEBUG:
        dbg_seq = pers.tile([128, C, BL, T], F32, tag="dbg_seq")
        nc.vector.tensor_copy(out=dbg_seq[:], in_=seq8[:])
        nc.sync.dma_start(out=dbg["seq"], in_=dbg_seq[:])

    # =================== STAGE 3: utterance attention ===================
    seq8v = seq8[:].rearrange("p c b t -> p c (b t)")
    with tc.tile_pool(name="ps_su", bufs=2, space="PSUM") as ps_su, \
         tc.tile_pool(name="ps_scu", bufs=1, space="PSUM") as ps_scu, \
         tc.tile_pool(name="s3tmp", bufs=2) as s3tmp:
        su8 = s3tmp.tile([128, C, BL, T], FP8, tag="su8")
        for half in range(2):
            psu = ps_su.tile([128, 4, BL, T], F32, tag="psu")
            for mm in range(4):
                m = half * 4 + mm
                for kp in range(CP):
                    nc.tensor.matmul(
                        out=psu[:, mm],
                        lhsT=w1u_s[:, 2 * kp:2 * kp + 2,
                                   m * 128:(m + 1) * 128],
                        rhs=seq8v[:, 2 * kp:2 * kp + 2],
                        start=(kp == 0), stop=(kp == CP - 1), perf_mode=DR)
            qn = s3tmp.tile([128, 4, BL, T], F32, tag="qn")
            nc.vector.tensor_tensor(
                out=qn[:], in0=psu[:],
                in1=_bcast_last(qu_s[:, half * 4:half * 4 + 4], T), op=OP.add)
            nc.scalar.activation(out=su8[:, half * 4:half * 4 + 4],
                                 in_=qn[:], func=AF.Tanh)
        pscu = ps_scu.tile([128, BL, T], F32)
        for cp in range(CP):
            nc.tensor.matmul(out=pscu[:], lhsT=vu_s[:, 2 * cp:2 * cp + 2],
                             rhs=su8[:, 2 * cp:2 * cp + 2],
                             start=(cp == 0), stop=(cp == CP - 1),
                             perf_mode=DR)
        eu = s3tmp.tile([128, BL, T], BF16, tag="eu")
        nc.scalar.activation(out=eu[:], in_=pscu[:], func=AF.Exp)
        rsu = s3tmp.tile([128, BL], F32, tag="rsu")
        nc.vector.reduce_sum(out=rsu[:], in_=eu[:], axis=AX.X)
        rcu = s3tmp.tile([128, BL], F32, tag="rcu")
        nc.vector.reciprocal(out=rcu[:], in_=rsu[:])
        pru = s3tmp.tile([128, C, BL, T], BF16, tag="pru")
        nc.vector.tensor_tensor(out=pru[:], in0=seq8[:],
                                in1=_bcast_mid(eu[:], C), op=OP.mult)
        redu = s3tmp.tile([128, C, BL], F32, tag="redu")
        nc.vector.reduce_sum(out=redu[:], in_=pru[:], axis=AX.X)
        nc.vector.tensor_tensor(out=ctxv8[:], in0=redu[:],
                                in1=_bcast_mid(rcu[:], C), op=OP.mult)
    if DEBUG:
        dbg_cv = pers.tile([128, C, BL], F32, tag="dbg_cv")
        nc.vector.tensor_copy(out=dbg_cv[:], in_=ctxv8[:])
        nc.sync.dma_start(out=dbg["ctxv"], in_=dbg_cv[:])

    # =================== STAGE 4: decoder GRU step ===================
    with tc.tile_pool(name="ps_xmd", bufs=2, space="PSUM") as ps_xmd, \
         tc.tile_pool(name="s4tmp", bufs=1) as s4tmp:
        # emb-half of the input kernel (deckB) + dec bias; runs as soon as
        # deckB lands (before ctxv is ready)
        pxB = ps_xmd.tile([128, 3, C, BL], F32, tag="pxB")
        for g in range(3):
            for c in range(C):
                col0 = g * D + c * 128
                for kp in range(CP):
                    nc.tensor.matmul(
                        out=pxB[:, g, c],
                        lhsT=deckB_s[:, 2 * kp:2 * kp + 2, col0:col0 + 128],
                        rhs=xembT_8[:, 2 * kp:2 * kp + 2],
                        start=(kp == 0), stop=False, perf_mode=DR)
                nc.tensor.matmul(out=pxB[:, g, c],
                                 lhsT=dbx_s[:, col0:col0 + 128],
                                 rhs=ones_s[:, :BL], start=False, stop=True)
        # bh = hmd + xmdB (both ready before ctxv); keep an SBUF copy of the
        # h-slice (a TensorTensor may read at most one PSUM operand)
        xB_h = s4tmp.tile([128, C, BL], F32, tag="xB_h")
        nc.vector.tensor_copy(out=xB_h[:], in_=pxB[:, 2])
        nc.vector.tensor_tensor(out=bh_sb[:], in0=hmd_sb[:], in1=pxB[:],
                                op=OP.add)
        if DEBUG:
            nc.sync.dma_start(out=dbg["hmd"], in_=hmd_sb[:])

        # ctxv-half (deckA)
        pxA = ps_xmd.tile([128, 3, C, BL], F32, tag="pxA")
        for g in range(3):
            for c in range(C):
                col0 = g * D + c * 128
                for kp in range(CP):
                    nc.tensor.matmul(
                        out=pxA[:, g, c],
                        lhsT=deckA_s[:, 2 * kp:2 * kp + 2, col0:col0 + 128],
                        rhs=ctxv8[:, 2 * kp:2 * kp + 2],
                        start=(kp == 0), stop=(kp == CP - 1), perf_mode=DR)
        if DEBUG:
            dbg_xm = s4tmp.tile([128, 3, C, BL], F32, tag="dbg_xm")
            nc.vector.tensor_copy(out=dbg_xm[:], in_=pxB[:])
            nc.vector.tensor_tensor(out=dbg_xm[:], in0=dbg_xm[:], in1=pxA[:],
                                    op=OP.add)
            nc.sync.dma_start(out=dbg["xmd"], in_=dbg_xm[:])

        tz = s4tmp.tile([128, C, BL], F32, tag="tz4")
        tr = s4tmp.tile([128, C, BL], F32, tag="tr4")
        rin = s4tmp.tile([128, C, BL], F32, tag="rin4")
        nc.vector.tensor_tensor(out=rin[:], in0=pxA[:, 1], in1=bh_sb[:, 1],
                                op=OP.add)
        nc.scalar.activation(out=tr[:], in_=rin[:], func=AF.Tanh, scale=0.5)
        zin = s4tmp.tile([128, C, BL], F32, tag="zin4")
        nc.vector.tensor_tensor(out=zin[:], in0=pxA[:, 0], in1=bh_sb[:, 0],
                                op=OP.add)
        nc.scalar.activation(out=tz[:], in_=zin[:], func=AF.Tanh, scale=0.5)
        # candidate: cin/2 = xh + r*hh with xh = xA_h + xB_h + b0_h (host-
        # doubled cols/bias), hh = hmd_h + b1_h (plain). bh[2] mixes hmd_h
        # into the x-side, so use pxB[2] and hmd_sb[2] directly here.
        rhh = s4tmp.tile([128, C, BL], F32, tag="rhh4")
        nc.vector.scalar_tensor_tensor(out=rhh[:], in0=tr[:], scalar=1.0,
                                       in1=hmd_sb[:, 2], op0=OP.add,
                                       op1=OP.mult)
        xh = s4tmp.tile([128, C, BL], F32, tag="xh4")
        nc.vector.tensor_tensor(out=xh[:], in0=pxA[:, 2], in1=xB_h[:],
                                op=OP.add)
        cin = s4tmp.tile([128, C, BL], F32, tag="cin4")
        nc.vector.tensor_tensor(out=cin[:], in0=xh[:], in1=rhh[:], op=OP.add)
        cand = s4tmp.tile([128, C, BL], F32, tag="cand4")
        nc.scalar.activation(out=cand[:], in_=cin[:], func=AF.Tanh, scale=0.5)
        zcm = s4tmp.tile([128, C, BL], F32, tag="zcm4")
        nc.vector.tensor_scalar(out=zcm[:], in0=tz[:], scalar1=-1.0,
                                scalar2=-0.5, op0=OP.add, op1=OP.mult)
        d1 = s4tmp.tile([128, C, BL], F32, tag="d14")
        nc.vector.tensor_tensor(out=d1[:], in0=cand[:], in1=hidT_f[:],
                                op=OP.subtract)
        d2 = s4tmp.tile([128, C, BL], F32, tag="d24")
        nc.vector.tensor_tensor(out=d2[:], in0=d1[:], in1=zcm[:], op=OP.mult)
        stT = s4tmp.tile([128, C, BL], F32, tag="stT")
        nc.vector.tensor_tensor(out=stT[:], in0=hidT_f[:], in1=d2[:],
                                op=OP.add)
        nc.sync.dma_start(out=ins["out"], in_=stT[:])

    es.close()


# ---------------------------------------------------------------------------
# Host side
# ---------------------------------------------------------------------------

_NC_CACHE = {}


def _get_nc():
    key = ("prog_v2", DEBUG)
    if key not in _NC_CACHE:
        _NC_CACHE[key] = build()
    return _NC_CACHE[key]


def _f8(a):
    return np.ascontiguousarray(np.asarray(a, np.float32)
                                .astype(ml_dtypes.float8_e4m3fn))


def _bf(a):
    return np.ascontiguousarray(np.asarray(a, np.float32)
                                .astype(ml_dtypes.bfloat16))


def _f32(a):
    return np.ascontiguousarray(np.asarray(a, np.float32))


def _chunked_T(w):
    """[D_in, N] -> [128, D_in//128, N]: row-chunked per-k lhsT tiles."""
    d_in, n = w.shape
    return np.ascontiguousarray(w.reshape(d_in // 128, 128, n)
                                .transpose(1, 0, 2))


def prepare_in_maps(inputs):
    x = np.asarray(inputs["x"]).astype(np.int64).reshape(B)
    hidden = _f32(inputs["hidden"])
    enc = _f32(inputs["encoder_outputs"])          # [64, 10, 50, 1024]
    maskf = np.asarray(inputs["context_mask"]).astype(np.float32)
    emb = np.asarray(inputs["embed_table"])

    x_emb = emb[x].astype(np.float32)

    def tmajor(a2d):  # [B, D] -> [128, C, B]
        return np.ascontiguousarray(
            a2d.T.reshape(C, 128, a2d.shape[0]).transpose(1, 0, 2))

    def dbl_h(w):
        w = np.array(w, np.float32, copy=True)
        w[:, 2 * D:] *= 2.0
        return w

    w1w = _f8(_chunked_T(np.asarray(inputs["w1_word"], np.float32)))
    w2w = _f8(_chunked_T(np.asarray(inputs["w2_word"], np.float32)))
    w1u = _f8(_chunked_T(np.asarray(inputs["w1_utt"], np.float32)))
    w2u = _f8(_chunked_T(np.asarray(inputs["w2_utt"], np.float32)))
    ctxk = _f8(_chunked_T(dbl_h(np.asarray(inputs["ctx_kernel"], np.float32))))
    ctxrk = _f8(_chunked_T(np.asarray(inputs["ctx_rec_kernel"], np.float32)))
    deck_full = dbl_h(np.asarray(inputs["dec_kernel"], np.float32))
    deckA = _f8(_chunked_T(deck_full[:D]))
    deckB = _f8(_chunked_T(deck_full[D:]))
    decrk = _bf(np.asarray(inputs["dec_rec_kernel"], np.float32)
                .reshape(C, 128, G3))

    def vrep(v):
        vc = np.asarray(v, np.float32).reshape(C, 128).T
        return _f8(np.broadcast_to(vc[:, :, None], (128, C, 128)))

    vw = vrep(inputs["v_word"])
    vu = vrep(inputs["v_utt"])

    def mchunk(v):
        return _f32(np.asarray(v, np.float32).reshape(C, 128).T)

    qb_w = mchunk(np.asarray(inputs["b1_word"], np.float32)
                  + np.asarray(inputs["b2_word"], np.float32))
    qb_u = mchunk(np.asarray(inputs["b1_utt"], np.float32)
                  + np.asarray(inputs["b2_utt"], np.float32))

    cbias = np.asarray(inputs["ctx_bias"], np.float32)
    dbias = np.asarray(inputs["dec_bias"], np.float32)

    def gate_bias_row(bias2):
        return np.concatenate([
            bias2[0, :D] + bias2[1, :D],
            bias2[0, D:2 * D] + bias2[1, D:2 * D],
            2.0 * bias2[0, 2 * D:],
        ]).reshape(1, G3)

    cbx = _bf(gate_bias_row(cbias))
    dbx = _bf(gate_bias_row(dbias))
    cb1h = _bf(cbias[1, 2 * D:].reshape(1, D))
    db1h = _bf(dbias[1, 2 * D:].reshape(1, D))

    ones_b = _bf(np.ones((1, BL * T), np.float32))

    enc_r = enc.reshape(B, R, D)

    in_maps = []
    for core in range(NCORES):
        sl = slice(core * BL, (core + 1) * BL)
        enc_t = np.ascontiguousarray(
            enc_r[sl].transpose(0, 2, 1)
            .reshape(BL, C, 128, R)
            .transpose(0, 2, 1, 3))
        hid_c = hidden[sl]
        mask_t = np.ascontiguousarray(
            np.broadcast_to(-0.5 * maskf[sl].T[None, :, :], (128, T, BL)))
        in_maps.append({
            "enc_t": _f8(enc_t),
            "hidT_f": _f32(tmajor(hid_c)),
            "hidT_b": _bf(tmajor(hid_c)),
            "hidT_8": _f8(tmajor(hid_c)),
            "xembT_8": _f8(tmajor(x_emb[sl])),
            "w1w": w1w, "w2w": w2w, "vw_rep": vw,
            "w1u": w1u, "w2u": w2u, "vu_rep": vu,
            "ctxk": ctxk, "ctxrk": ctxrk,
            "deckA": deckA, "deckB": deckB, "decrk": decrk,
            "qb_w": qb_w, "qb_u": qb_u,
            "cbx_row": cbx, "cb1h_b": cb1h,
            "dbx_row": dbx, "db1h_b": db1h,
            "mask_t": _f32(mask_t),
            "ones_b": ones_b,
        })
    return in_maps


def run(inputs):
    nc = _get_nc()
    in_maps = prepare_in_maps(inputs)
    res = run_bass_kernel_spmd(nc, in_maps, list(range(NCORES)))
    # out per core: [128, C, BL] feature-major; host transposes to [BL, D]
    parts = []
    for c in range(NCORES):
        o = np.asarray(res.results[c]["out"])           # [128, C, BL]
        parts.append(o.transpose(2, 1, 0).reshape(BL, D))
    out = np.concatenate(parts, axis=0)
    return np.ascontiguousarray(out.astype(np.float32)), res


def kernel(**inputs):
    out, _ = run(inputs)
    return out, out


# revision 32
# speedup vs baseline: 1.7448x; 1.1024x over previous
"""Trainium2 Bass kernel for nn_Decoder_55688545960558 (v2, fp8).

Hierarchical-attention GRU decoder step, data-parallel over batch
(64 -> 8 per core), no collectives.

Key differences vs v1:
- All attention weights, enc, and the context-GRU weights in fp8e4
  (validated: rel_err ~2e-3); decoder GRU's recurrent kernel stays bf16.
  fp8 matmuls use DoubleRow perf mode (2 k-chunks per instruction).
- Word-attention softmax: exp is NOT normalized; the weighted sums are
  scaled by 1/Z at the end (saves a [128,500] DVE pass per batch).
- The weighted-sum multiplies are split DVE/GpSimd; reduce on DVE.
- All gate biases enter PSUM via rank-1 ones-matmuls; PSUM->SBUF moves
  are single wide ops (no per-chunk scalar adds).
- Output stays feature-major [128, C, BL]; the host transposes.
"""

from contextlib import ExitStack

import numpy as np
import ml_dtypes

import concourse.bass as bass
import concourse.mybir as mybir
import concourse.tile as tile
from concourse import bacc
from concourse.bass_utils import run_bass_kernel_spmd

F32 = mybir.dt.float32
BF16 = mybir.dt.bfloat16
FP8 = mybir.dt.float8e4
AF = mybir.ActivationFunctionType
OP = mybir.AluOpType
AX = mybir.AxisListType
DR = mybir.MatmulPerfMode.DoubleRow

NCORES = 8
B = 64
BL = B // NCORES  # 8
T = 10
S = 50
R = T * S         # 500
D = 1024
U = 1024
C = D // 128      # 8
CP = C // 2       # 4 k-pairs for DoubleRow
G3 = 3 * D        # 3072

DEBUG = False


def _bcast_mid(ap, n):
    """Insert a 0-stride broadcast dim of size n as dim 1 (after partitions)."""
    return bass.AP(tensor=ap.tensor, offset=ap.offset,
                   ap=[ap.ap[0], [0, n]] + list(ap.ap[1:]))


def _bcast_last(ap, n):
    return bass.AP(tensor=ap.tensor, offset=ap.offset,
                   ap=list(ap.ap) + [[0, n]])


def build():
    nc = bacc.Bacc("TRN2", target_bir_lowering=False, debug=False,
                   num_devices=NCORES)

    def din(name, shape, dt):
        return nc.dram_tensor(name, list(shape), dt, kind="ExternalInput").ap()

    ins = {}
    ins["enc"] = din("enc_t", [BL, 128, C, R], FP8)
    ins["hidT_f"] = din("hidT_f", [128, C, BL], F32)
    ins["hidT_b"] = din("hidT_b", [128, C, BL], BF16)
    ins["hidT_8"] = din("hidT_8", [128, C, BL], FP8)
    ins["xembT_8"] = din("xembT_8", [128, C, BL], FP8)
    ins["w1w"] = din("w1w", [128, C, U], FP8)
    ins["w2w"] = din("w2w", [128, C, U], FP8)
    ins["vw"] = din("vw_rep", [128, C, 128], FP8)
    ins["w1u"] = din("w1u", [128, C, U], FP8)
    ins["w2u"] = din("w2u", [128, C, U], FP8)
    ins["vu"] = din("vu_rep", [128, C, 128], FP8)
    ins["ctxk"] = din("ctxk", [128, C, G3], FP8)
    ins["ctxrk"] = din("ctxrk", [128, C, G3], FP8)
    ins["deckA"] = din("deckA", [128, C, G3], FP8)
    ins["deckB"] = din("deckB", [128, C, G3], FP8)
    ins["decrk"] = din("decrk", [C, 128, G3], BF16)
    ins["qb_w"] = din("qb_w", [128, C], F32)
    ins["qb_u"] = din("qb_u", [128, C], F32)
    ins["cbx_row"] = din("cbx_row", [1, G3], BF16)
    ins["cb1h"] = din("cb1h_b", [1, D], BF16)
    ins["dbx_row"] = din("dbx_row", [1, G3], BF16)
    ins["db1h"] = din("db1h_b", [1, D], BF16)
    ins["mask"] = din("mask_t", [128, T, BL], F32)   # pre-scaled by -0.5
    ins["ones"] = din("ones_b", [1, BL * T], BF16)

    ins["out"] = nc.dram_tensor("out", [128, C, BL], F32,
                                kind="ExternalOutput").ap()
    dbg = {}
    if DEBUG:
        dbg["q"] = nc.dram_tensor("dbg_q", [128, C, BL], F32,
                                  kind="ExternalOutput").ap()
        dbg["ctx"] = nc.dram_tensor("dbg_ctx", [128, C, BL, T], F32,
                                    kind="ExternalOutput").ap()
        dbg["seq"] = nc.dram_tensor("dbg_seq", [128, C, BL, T], F32,
                                    kind="ExternalOutput").ap()
        dbg["ctxv"] = nc.dram_tensor("dbg_ctxv", [128, C, BL], F32,
                                     kind="ExternalOutput").ap()
        dbg["hmd"] = nc.dram_tensor("dbg_hmd", [128, 3, C, BL], F32,
                                    kind="ExternalOutput").ap()
        dbg["xmd"] = nc.dram_tensor("dbg_xmd", [128, 3, C, BL], F32,
                                    kind="ExternalOutput").ap()
    ins["dbg"] = dbg

    with nc.allow_low_precision(reason="bf16/fp8 activations by design"):
        with tile.TileContext(nc) as tc:
            _emit(nc, tc, ins)
    nc.compile()
    return nc


def _emit(nc, tc, ins):
    dbg = ins["dbg"]
    es = ExitStack()

    pers = es.enter_context(tc.tile_pool(name="pers", bufs=1))
    wsA = es.enter_context(tc.tile_pool(name="wsA", bufs=1))    # w1w -> w1u
    wsB = es.enter_context(tc.tile_pool(name="wsB", bufs=1))    # w2w -> w2u
    gruw = es.enter_context(tc.tile_pool(name="gruw", bufs=1))  # ctxk/ctxrk
    decw = es.enter_context(tc.tile_pool(name="decw", bufs=1))  # deckA/deckB
    encp = es.enter_context(tc.tile_pool(name="encp", bufs=3))
    drkp = es.enter_context(tc.tile_pool(name="drkp", bufs=4))
    thp = es.enter_context(tc.tile_pool(name="thp", bufs=1))
    ep = es.enter_context(tc.tile_pool(name="ep", bufs=2))
    prp = es.enter_context(tc.tile_pool(name="prp", bufs=3))
    s1small = es.enter_context(tc.tile_pool(name="s1small", bufs=2))
    gtmp = es.enter_context(tc.tile_pool(name="gtmp", bufs=2))
    hstate = es.enter_context(tc.tile_pool(name="hstate", bufs=2))
    s34 = es.enter_context(tc.tile_pool(name="s34", bufs=2))

    def ld(pool, dram_ap, shape, dt, name, chunked=False):
        t = pool.tile(list(shape), dt, tag=name, name=name)
        if chunked:
            for k in range(shape[1]):
                nc.sync.dma_start(out=t[:, k], in_=dram_ap[:, k])
        else:
            nc.sync.dma_start(out=t[:], in_=dram_ap)
        return t

    # ---------------- DMA: critical-path order on the sync queue ----------
    w1w_s = ld(wsA, ins["w1w"], [128, C, U], FP8, "wA")
    enc_tiles = [ld(encp, ins["enc"][0], [128, C, R], FP8, "enc")]
    hidT_8 = ld(pers, ins["hidT_8"], [128, C, BL], FP8, "hidT_8")
    qbw_s = ld(pers, ins["qb_w"], [128, C], F32, "qbw")
    w2w_s = ld(wsB, ins["w2w"], [128, C, U], FP8, "wB")
    vw_s = ld(pers, ins["vw"], [128, C, 128], FP8, "vw")
    for b in range(1, BL):
        enc_tiles.append(ld(encp, ins["enc"][b], [128, C, R], FP8, "enc"))
    hidT_f = ld(pers, ins["hidT_f"], [128, C, BL], F32, "hidT_f")
    hidT_b = ld(pers, ins["hidT_b"], [128, C, BL], BF16, "hidT_b")
    xembT_8 = ld(pers, ins["xembT_8"], [128, C, BL], FP8, "xembT_8")
    qbu_s = ld(pers, ins["qb_u"], [128, C], F32, "qbu")
    mask_s = ld(pers, ins["mask"], [128, T, BL], F32, "mask")
    ones_s = ld(pers, ins["ones"], [1, BL * T], BF16, "ones")
    cbx_s = ld(pers, ins["cbx_row"], [1, G3], BF16, "cbx")
    cb1h_s = ld(pers, ins["cb1h"], [1, D], BF16, "cb1h")
    dbx_s = ld(pers, ins["dbx_row"], [1, G3], BF16, "dbx")
    db1h_s = ld(pers, ins["db1h"], [1, D], BF16, "db1h")
    ctxk_s = ld(gruw, ins["ctxk"], [128, C, G3], FP8, "ctxk")
    ctxrk_s = ld(gruw, ins["ctxrk"], [128, C, G3], FP8, "ctxrk")
    w1u_s = ld(wsA, ins["w1u"], [128, C, U], FP8, "wA")
    w2u_s = ld(wsB, ins["w2u"], [128, C, U], FP8, "wB")
    vu_s = ld(pers, ins["vu"], [128, C, 128], FP8, "vu")
    # decrk in 4 pair-DMAs so hm_dec can stream during the GRU scan
    decrk_tiles = []
    for j in range(C // 2):
        dk = drkp.tile([128, 2, G3], BF16, tag="drk", name=f"decrk{j}")
        nc.sync.dma_start(
            out=dk[:],
            in_=ins["decrk"][2 * j:2 * j + 2].rearrange("c p g -> p c g"))
        decrk_tiles.append(dk)
    # deckA rotates into deckB's slot (deckB is consumed by the xmdB
    # precompute before deckA's transfer may land)
    deckB_s = decw.tile([128, C, G3], FP8, tag="deck", name="deckB")
    nc.sync.dma_start(out=deckB_s[:], in_=ins["deckB"])
    deckA_s = decw.tile([128, C, G3], FP8, tag="deck", name="deckA")
    nc.sync.dma_start(out=deckA_s[:], in_=ins["deckA"])

    # cross-stage activations
    qsb = pers.tile([128, C, BL], F32, tag="qsb")
    qu_s = pers.tile([128, C, BL], F32, tag="qu")
    ctx8 = pers.tile([128, C, BL, T], FP8, tag="ctx8")
    seq8 = pers.tile([128, C, BL, T], FP8, tag="seq8")
    xg = [pers.tile([128, C, BL, T], FP8, tag=f"xg{g}", name=f"xg{g}")
          for g in range(3)]
    hmd_sb = pers.tile([128, 3, C, BL], F32, tag="hmd_sb")
    bh_sb = pers.tile([128, 3, C, BL], F32, tag="bh_sb")
    ctxv8 = pers.tile([128, C, BL], FP8, tag="ctxv8")

    # =================== STAGE 1: word attention ===================
    with tc.tile_pool(name="ps_score", bufs=5, space="PSUM") as p_score, \
         tc.tile_pool(name="pq", bufs=1, space="PSUM") as pq:
        def score_group(enc_b, m):
            ps = p_score.tile([128, R], F32, tag="ps")
            for kp in range(CP):
                nc.tensor.matmul(out=ps[:],
                                 lhsT=w1w_s[:, 2 * kp:2 * kp + 2,
                                            m * 128:(m + 1) * 128],
                                 rhs=enc_b[:, 2 * kp:2 * kp + 2],
                                 start=(kp == 0), stop=(kp == CP - 1),
                                 perf_mode=DR)
            return ps

        def q_matmuls(w_s, qb, out_sb):
            p_q = pq.tile([128, C, BL], F32, tag="pq")
            for mm in range(C):
                for kp in range(CP):
                    nc.tensor.matmul(out=p_q[:, mm],
                                     lhsT=w_s[:, 2 * kp:2 * kp + 2,
                                              mm * 128:(mm + 1) * 128],
                                     rhs=hidT_8[:, 2 * kp:2 * kp + 2],
                                     start=(kp == 0), stop=(kp == CP - 1),
                                     perf_mode=DR)
            for mm in range(C):
                nc.vector.tensor_scalar_add(out=out_sb[:, mm], in0=p_q[:, mm],
                                            scalar1=qb[:, mm:mm + 1])

        pending = None   # (pr, rc, b) of the previous batch

        def flush_pending():
            # reduce+scale for batch b-1, deferred so the in-order DVE queue
            # fills the wait on the Pool multiply with batch-b work
            nonlocal pending
            if pending is None:
                return
            pr_p, rc_p, b_p = pending
            red = s1small.tile([128, C, T], F32, tag="red")
            nc.vector.reduce_sum(out=red[:], in_=pr_p[:], axis=AX.X)
            nc.vector.tensor_tensor(out=ctx8[:, :, b_p, :], in0=red[:],
                                    in1=_bcast_mid(rc_p[:], C), op=OP.mult)
            pending = None

        for b in range(BL):
            enc_b = enc_tiles[b]
            th = thp.tile([128, C, R], FP8, tag="th")
            if b == 0:
                pss = [score_group(enc_b, m) for m in range(4)]
                q_matmuls(w2w_s, qbw_s, qsb)
                if DEBUG:
                    nc.sync.dma_start(out=dbg["q"], in_=qsb[:])
                for m in range(4):
                    nc.scalar.activation(out=th[:, m], in_=pss[m][:],
                                         func=AF.Tanh,
                                         bias=qsb[:, m, b:b + 1])
                for m in range(4, C):
                    ps = score_group(enc_b, m)
                    nc.scalar.activation(out=th[:, m], in_=ps[:],
                                         func=AF.Tanh,
                                         bias=qsb[:, m, b:b + 1])
            else:
                for m in range(C):
                    ps = score_group(enc_b, m)
                    nc.scalar.activation(out=th[:, m], in_=ps[:],
                                         func=AF.Tanh,
                                         bias=qsb[:, m, b:b + 1])
            # V matmul (replicated scores on all partitions)
            psc = p_score.tile([128, R], F32, tag="ps")
            for cp in range(CP):
                nc.tensor.matmul(out=psc[:], lhsT=vw_s[:, 2 * cp:2 * cp + 2],
                                 rhs=th[:, 2 * cp:2 * cp + 2],
                                 start=(cp == 0), stop=(cp == CP - 1),
                                 perf_mode=DR)
            e = s1small.tile([128, T, S], BF16, tag="e")
            nc.scalar.activation(
                out=e[:], in_=psc[:].rearrange("p (t s) -> p t s", s=S),
                func=AF.Exp)
            # unnormalized weighted sum: pr = enc * e  (DVE 2 chunks, Pool 6)
            pr = prp.tile([128, C, T, S], FP8, tag="pr")
            encv = enc_b[:].rearrange("p c (t s) -> p c t s", s=S)
            nc.vector.tensor_tensor(out=pr[:, 0:2], in0=encv[:, 0:2],
                                    in1=_bcast_mid(e[:], 2), op=OP.mult)
            nc.gpsimd.tensor_tensor(out=pr[:, 2:8], in0=encv[:, 2:8],
                                    in1=_bcast_mid(e[:], 6), op=OP.mult)
            rs = s1small.tile([128, T], F32, tag="rs")
            nc.vector.reduce_sum(out=rs[:], in_=e[:], axis=AX.X)
            rc = s1small.tile([128, T], F32, tag="rc")
            nc.vector.reciprocal(out=rc[:], in_=rs[:])
            flush_pending()
            pending = (pr, rc, b)
        flush_pending()
        # utt query after the batch loop: w2u's DMA lands late in the
        # stream, and the in-order PE queue must not stall stage 1 on it
        q_matmuls(w2u_s, qbu_s, qu_s)
    if DEBUG:
        dbg_ctx = pers.tile([128, C, BL, T], F32, tag="dbg_ctx")
        nc.vector.tensor_copy(out=dbg_ctx[:], in_=ctx8[:])
        nc.sync.dma_start(out=dbg["ctx"], in_=dbg_ctx[:])

    # =================== STAGE 2: context GRU ===================
    ctx8v = ctx8[:].rearrange("p c b t -> p c (b t)")
    with tc.tile_pool(name="ps_xm", bufs=3, space="PSUM") as ps_xm:
        for g in range(3):
            for half in range(2):
                pxm = ps_xm.tile([128, 4, BL, T], F32, tag="pxm")
                for cc in range(4):
                    c = half * 4 + cc
                    col0 = g * D + c * 128
                    for kp in range(CP):
                        nc.tensor.matmul(
                            out=pxm[:, cc],
                            lhsT=ctxk_s[:, 2 * kp:2 * kp + 2, col0:col0 + 128],
                            rhs=ctx8v[:, 2 * kp:2 * kp + 2],
                            start=(kp == 0), stop=False, perf_mode=DR)
                    # bias as rank-1 ones-matmul closes the group
                    nc.tensor.matmul(out=pxm[:, cc],
                                     lhsT=cbx_s[:, col0:col0 + 128],
                                     rhs=ones_s[:], start=False, stop=True)
                nc.vector.tensor_copy(out=xg[g][:, half * 4:half * 4 + 4],
                                      in_=pxm[:])

    h_f = None
    ps_hmd = es.enter_context(tc.tile_pool(name="ps_hmd", bufs=2, space="PSUM"))
    ps_xmd = es.enter_context(tc.tile_pool(name="ps_xmd", bufs=1, space="PSUM"))
    xmdB_sb = pers.tile([128, 3, C, BL], F32, tag="xmdB_sb")

    with tc.tile_pool(name="ps_hm", bufs=2, space="PSUM") as ps_hm:
        for t in range(T):
            phm = ps_hm.tile([128, 3, C, BL], F32, tag="phm")
            for g in (1, 2, 0):   # r first: it gates the candidate chain
                for c in range(C):
                    col0 = g * D + c * 128
                    if t > 0:
                        for kp in range(CP):
                            nc.tensor.matmul(
                                out=phm[:, g, c],
                                lhsT=ctxrk_s[:, 2 * kp:2 * kp + 2,
                                             col0:col0 + 128],
                                rhs=seq8[:, 2 * kp:2 * kp + 2, :, t - 1],
                                start=(kp == 0),
                                stop=(kp == CP - 1 and g != 2),
                                perf_mode=DR)
                    if g == 2:
                        nc.tensor.matmul(out=phm[:, g, c],
                                         lhsT=cb1h_s[:, c * 128:(c + 1) * 128],
                                         rhs=ones_s[:, :BL], start=(t == 0),
                                         stop=True)
            if t == 3:
                # emb-half of the decoder input kernel in a GRU PE gap
                # (deckB has landed; copying to SBUF frees its slot so the
                # in-order DMA queue can start deckA's transfer)
                pxB = ps_xmd.tile([128, 3, C, BL], F32, tag="pxB")
                for g in range(3):
                    for c in range(C):
                        col0 = g * D + c * 128
                        for kp in range(CP):
                            nc.tensor.matmul(
                                out=pxB[:, g, c],
                                lhsT=deckB_s[:, 2 * kp:2 * kp + 2,
                                             col0:col0 + 128],
                                rhs=xembT_8[:, 2 * kp:2 * kp + 2],
                                start=(kp == 0), stop=False, perf_mode=DR)
                        nc.tensor.matmul(out=pxB[:, g, c],
                                         lhsT=dbx_s[:, col0:col0 + 128],
                                         rhs=ones_s[:, :BL], start=False,
                                         stop=True)
            if t == 4:
                nc.vector.tensor_copy(out=xmdB_sb[:], in_=pxB[:])
            # sigmoid(x) == (tanh(x/2)+1)/2; affine parts folded on host
            mask_bc = _bcast_mid(mask_s[:, t, :], C)
            tz = gtmp.tile([128, C, BL], F32, tag="tz")
            tr = gtmp.tile([128, C, BL], F32, tag="tr")
            if t == 0:
                nc.scalar.activation(out=tr[:], in_=xg[1][:, :, :, 0],
                                     func=AF.Tanh, scale=0.5)
                nc.scalar.activation(out=tz[:], in_=xg[0][:, :, :, 0],
                                     func=AF.Tanh, scale=0.5)
            else:
                rin = gtmp.tile([128, C, BL], F32, tag="rin")
                nc.vector.tensor_tensor(out=rin[:], in0=xg[1][:, :, :, t],
                                        in1=phm[:, 1], op=OP.add)
                nc.scalar.activation(out=tr[:], in_=rin[:], func=AF.Tanh,
                                     scale=0.5)
                zin = gtmp.tile([128, C, BL], F32, tag="zin")
                nc.vector.tensor_tensor(out=zin[:], in0=xg[0][:, :, :, t],
                                        in1=phm[:, 0], op=OP.add)
                nc.scalar.activation(out=tz[:], in_=zin[:], func=AF.Tanh,
                                     scale=0.5)
            # rhh = (tanh_r + 1) * hh == 2*r*hh; xg-h host-doubled
            # rhh = (tanh_r + 1) * hh; at t==0 phm[2] is the h-bias only
            rhh = gtmp.tile([128, C, BL], F32, tag="rhh")
            nc.vector.scalar_tensor_tensor(out=rhh[:], in0=tr[:],
                                           scalar=1.0, in1=phm[:, 2],
                                           op0=OP.add, op1=OP.mult)
            cin = gtmp.tile([128, C, BL], F32, tag="cin")
            nc.vector.tensor_tensor(out=cin[:], in0=xg[2][:, :, :, t],
                                    in1=rhh[:], op=OP.add)
            # zcm = (1-z)*mask == (tanh_z - 1) * (-0.5*mask)
            zcm = gtmp.tile([128, C, BL], F32, tag="zcm")
            nc.vector.scalar_tensor_tensor(out=zcm[:], in0=tz[:], scalar=-1.0,
                                           in1=mask_bc, op0=OP.add,
                                           op1=OP.mult)
            h_f2 = hstate.tile([128, C, BL], F32, tag="h_f")
            if t > 0:
                hz1 = gtmp.tile([128, C, BL], F32, tag="hz1")
                nc.vector.tensor_tensor(out=hz1[:], in0=h_f[:], in1=zcm[:],
                                        op=OP.mult)
                hm1 = gtmp.tile([128, C, BL], F32, tag="hm1")
                nc.vector.tensor_tensor(out=hm1[:], in0=h_f[:], in1=hz1[:],
                                        op=OP.subtract)
            cand = gtmp.tile([128, C, BL], F32, tag="cand")
            nc.scalar.activation(out=cand[:], in_=cin[:], func=AF.Tanh,
                                 scale=0.5)
            if t == 0:
                nc.vector.tensor_tensor(out=seq8[:, :, :, 0], in0=cand[:],
                                        in1=zcm[:], op=OP.mult)
                nc.vector.tensor_tensor(out=h_f2[:], in0=cand[:], in1=zcm[:],
                                        op=OP.mult)
            else:
                t2 = gtmp.tile([128, C, BL], F32, tag="t2")
                nc.vector.tensor_tensor(out=t2[:], in0=cand[:], in1=zcm[:],
                                        op=OP.mult)
                nc.vector.tensor_tensor(out=seq8[:, :, :, t], in0=hm1[:],
                                        in1=t2[:], op=OP.add)
                nc.vector.tensor_tensor(out=h_f2[:], in0=hm1[:], in1=t2[:],
                                        op=OP.add)
            h_f = h_f2
    if DEBUG:
        dbg_seq = pers.tile([128, C, BL, T], F32, tag="dbg_seq")
        nc.vector.tensor_copy(out=dbg_seq[:], in_=seq8[:])
        nc.sync.dma_start(out=dbg["seq"], in_=dbg_seq[:])

    # =================== STAGE 3: utterance attention ===================
    seq8v = seq8[:].rearrange("p c b t -> p c (b t)")
    with tc.tile_pool(name="ps_su", bufs=2, space="PSUM") as ps_su, \
         tc.tile_pool(name="ps_scu", bufs=1, space="PSUM") as ps_scu, \
         tc.tile_pool(name="s3tmp", bufs=2) as s3tmp:
        su8 = s3tmp.tile([128, C, BL, T], FP8, tag="su8")
        for half in range(2):
            psu = ps_su.tile([128, 4, BL, T], F32, tag="psu")
            for mm in range(4):
                m = half * 4 + mm
                for kp in range(CP):
                    nc.tensor.matmul(
                        out=psu[:, mm],
                        lhsT=w1u_s[:, 2 * kp:2 * kp + 2,
                                   m * 128:(m + 1) * 128],
                        rhs=seq8v[:, 2 * kp:2 * kp + 2],
                        start=(kp == 0), stop=(kp == CP - 1), perf_mode=DR)
            qn = s3tmp.tile([128, 4, BL, T], F32, tag="qn")
            nc.vector.tensor_tensor(
                out=qn[:], in0=psu[:],
                in1=_bcast_last(qu_s[:, half * 4:half * 4 + 4], T), op=OP.add)
            nc.scalar.activation(out=su8[:, half * 4:half * 4 + 4],
                                 in_=qn[:], func=AF.Tanh)
        # hm_dec = hidden @ dec_rec_kernel on PE under the stage-3
        # softmax's DVE/ACT shadow (decrk pairs have landed by now)
        for k in range(C):
            dk = decrk_tiles[k // 2][:, k % 2]
            phmd_k = ps_hmd.tile([128, 3, C, BL], F32, tag="phmd_k")
            for g in range(3):
                for c in range(C):
                    col0 = g * D + c * 128
                    nc.tensor.matmul(out=phmd_k[:, g, c],
                                     lhsT=dk[:, col0:col0 + 128],
                                     rhs=hidT_b[:, k], start=True,
                                     stop=(k != C - 1 or g != 2))
                    if k == C - 1 and g == 2:
                        nc.tensor.matmul(
                            out=phmd_k[:, g, c],
                            lhsT=db1h_s[:, c * 128:(c + 1) * 128],
                            rhs=ones_s[:, :BL], start=False, stop=True)
            if k == 0:
                nc.vector.tensor_copy(out=hmd_sb[:], in_=phmd_k[:])
            else:
                nc.vector.tensor_tensor(out=hmd_sb[:], in0=hmd_sb[:],
                                        in1=phmd_k[:], op=OP.add)
        pscu = ps_scu.tile([128, BL, T], F32)
        for cp in range(CP):
            nc.tensor.matmul(out=pscu[:], lhsT=vu_s[:, 2 * cp:2 * cp + 2],
                             rhs=su8[:, 2 * cp:2 * cp + 2],
                             start=(cp == 0), stop=(cp == CP - 1),
                             perf_mode=DR)
        eu = s3tmp.tile([128, BL, T], BF16, tag="eu")
        nc.scalar.activation(out=eu[:], in_=pscu[:], func=AF.Exp)
        rsu = s3tmp.tile([128, BL], F32, tag="rsu")
        nc.vector.reduce_sum(out=rsu[:], in_=eu[:], axis=AX.X)
        rcu = s3tmp.tile([128, BL], F32, tag="rcu")
        nc.vector.reciprocal(out=rcu[:], in_=rsu[:])
        pru = s3tmp.tile([128, C, BL, T], BF16, tag="pru")
        nc.vector.tensor_tensor(out=pru[:], in0=seq8[:],
                                in1=_bcast_mid(eu[:], C), op=OP.mult)
        redu = s3tmp.tile([128, C, BL], F32, tag="redu")
        nc.vector.reduce_sum(out=redu[:], in_=pru[:], axis=AX.X)
        nc.vector.tensor_tensor(out=ctxv8[:], in0=redu[:],
                                in1=_bcast_mid(rcu[:], C), op=OP.mult)
    if DEBUG:
        dbg_cv = pers.tile([128, C, BL], F32, tag="dbg_cv")
        nc.vector.tensor_copy(out=dbg_cv[:], in_=ctxv8[:])
        nc.sync.dma_start(out=dbg["ctxv"], in_=dbg_cv[:])

    # =================== STAGE 4: decoder GRU step ===================
    with tc.tile_pool(name="s4tmp", bufs=1) as s4tmp:
        nc.vector.tensor_tensor(out=bh_sb[:], in0=hmd_sb[:], in1=xmdB_sb[:],
                                op=OP.add)
        if DEBUG:
            nc.sync.dma_start(out=dbg["hmd"], in_=hmd_sb[:])

        # ctxv-half (deckA)
        pxA = ps_xmd.tile([128, 3, C, BL], F32, tag="pxA")
        for g in range(3):
            for c in range(C):
                col0 = g * D + c * 128
                for kp in range(CP):
                    nc.tensor.matmul(
                        out=pxA[:, g, c],
                        lhsT=deckA_s[:, 2 * kp:2 * kp + 2, col0:col0 + 128],
                        rhs=ctxv8[:, 2 * kp:2 * kp + 2],
                        start=(kp == 0), stop=(kp == CP - 1), perf_mode=DR)
        if DEBUG:
            dbg_xm = s4tmp.tile([128, 3, C, BL], F32, tag="dbg_xm")
            nc.vector.tensor_tensor(out=dbg_xm[:], in0=xmdB_sb[:],
                                    in1=pxA[:], op=OP.add)
            nc.sync.dma_start(out=dbg["xmd"], in_=dbg_xm[:])

        tz = s4tmp.tile([128, C, BL], F32, tag="tz4")
        tr = s4tmp.tile([128, C, BL], F32, tag="tr4")
        rin = s4tmp.tile([128, C, BL], F32, tag="rin4")
        nc.vector.tensor_tensor(out=rin[:], in0=pxA[:, 1], in1=bh_sb[:, 1],
                                op=OP.add)
        nc.scalar.activation(out=tr[:], in_=rin[:], func=AF.Tanh, scale=0.5)
        zin = s4tmp.tile([128, C, BL], F32, tag="zin4")
        nc.vector.tensor_tensor(out=zin[:], in0=pxA[:, 0], in1=bh_sb[:, 0],
                                op=OP.add)
        nc.scalar.activation(out=tz[:], in_=zin[:], func=AF.Tanh, scale=0.5)
        # candidate: cin/2 = xh + r*hh with xh = xA_h + xB_h + b0_h (host-
        # doubled cols/bias), hh = hmd_h + b1_h (plain). bh[2] mixes hmd_h
        # into the x-side, so use pxB[2] and hmd_sb[2] directly here.
        rhh = s4tmp.tile([128, C, BL], F32, tag="rhh4")
        nc.vector.scalar_tensor_tensor(out=rhh[:], in0=tr[:], scalar=1.0,
                                       in1=hmd_sb[:, 2], op0=OP.add,
                                       op1=OP.mult)
        xh = s4tmp.tile([128, C, BL], F32, tag="xh4")
        nc.vector.tensor_tensor(out=xh[:], in0=pxA[:, 2], in1=xmdB_sb[:, 2],
                                op=OP.add)
        cin = s4tmp.tile([128, C, BL], F32, tag="cin4")
        nc.vector.tensor_tensor(out=cin[:], in0=xh[:], in1=rhh[:], op=OP.add)
        cand = s4tmp.tile([128, C, BL], F32, tag="cand4")
        nc.scalar.activation(out=cand[:], in_=cin[:], func=AF.Tanh, scale=0.5)
        zcm = s4tmp.tile([128, C, BL], F32, tag="zcm4")
        nc.vector.tensor_scalar(out=zcm[:], in0=tz[:], scalar1=-1.0,
                                scalar2=-0.5, op0=OP.add, op1=OP.mult)
        d1 = s4tmp.tile([128, C, BL], F32, tag="d14")
        nc.vector.tensor_tensor(out=d1[:], in0=cand[:], in1=hidT_f[:],
                                op=OP.subtract)
        d2 = s4tmp.tile([128, C, BL], F32, tag="d24")
        nc.vector.tensor_tensor(out=d2[:], in0=d1[:], in1=zcm[:], op=OP.mult)
        stT = s4tmp.tile([128, C, BL], F32, tag="stT")
        nc.vector.tensor_tensor(out=stT[:], in0=hidT_f[:], in1=d2[:],
                                op=OP.add)
        nc.sync.dma_start(out=ins["out"], in_=stT[:])

    es.close()


# ---------------------------------------------------------------------------
# Host side
# ---------------------------------------------------------------------------

_NC_CACHE = {}


def _get_nc():
    key = ("prog_v2", DEBUG)
    if key not in _NC_CACHE:
        _NC_CACHE[key] = build()
    return _NC_CACHE[key]


def _f8(a):
    return np.ascontiguousarray(np.asarray(a, np.float32)
                                .astype(ml_dtypes.float8_e4m3fn))


def _bf(a):
    return np.ascontiguousarray(np.asarray(a, np.float32)
                                .astype(ml_dtypes.bfloat16))


def _f32(a):
    return np.ascontiguousarray(np.asarray(a, np.float32))


def _chunked_T(w):
    """[D_in, N] -> [128, D_in//128, N]: row-chunked per-k lhsT tiles."""
    d_in, n = w.shape
    return np.ascontiguousarray(w.reshape(d_in // 128, 128, n)
                                .transpose(1, 0, 2))


def prepare_in_maps(inputs):
    x = np.asarray(inputs["x"]).astype(np.int64).reshape(B)
    hidden = _f32(inputs["hidden"])
    enc = _f32(inputs["encoder_outputs"])          # [64, 10, 50, 1024]
    maskf = np.asarray(inputs["context_mask"]).astype(np.float32)
    emb = np.asarray(inputs["embed_table"])

    x_emb = emb[x].astype(np.float32)

    def tmajor(a2d):  # [B, D] -> [128, C, B]
        return np.ascontiguousarray(
            a2d.T.reshape(C, 128, a2d.shape[0]).transpose(1, 0, 2))

    def dbl_h(w):
        w = np.array(w, np.float32, copy=True)
        w[:, 2 * D:] *= 2.0
        return w

    w1w = _f8(_chunked_T(np.asarray(inputs["w1_word"], np.float32)))
    w2w = _f8(_chunked_T(np.asarray(inputs["w2_word"], np.float32)))
    w1u = _f8(_chunked_T(np.asarray(inputs["w1_utt"], np.float32)))
    w2u = _f8(_chunked_T(np.asarray(inputs["w2_utt"], np.float32)))
    ctxk = _f8(_chunked_T(dbl_h(np.asarray(inputs["ctx_kernel"], np.float32))))
    ctxrk = _f8(_chunked_T(np.asarray(inputs["ctx_rec_kernel"], np.float32)))
    deck_full = dbl_h(np.asarray(inputs["dec_kernel"], np.float32))
    deckA = _f8(_chunked_T(deck_full[:D]))
    deckB = _f8(_chunked_T(deck_full[D:]))
    decrk = _bf(np.asarray(inputs["dec_rec_kernel"], np.float32)
                .reshape(C, 128, G3))

    def vrep(v):
        vc = np.asarray(v, np.float32).reshape(C, 128).T
        return _f8(np.broadcast_to(vc[:, :, None], (128, C, 128)))

    vw = vrep(inputs["v_word"])
    vu = vrep(inputs["v_utt"])

    def mchunk(v):
        return _f32(np.asarray(v, np.float32).reshape(C, 128).T)

    qb_w = mchunk(np.asarray(inputs["b1_word"], np.float32)
                  + np.asarray(inputs["b2_word"], np.float32))
    qb_u = mchunk(np.asarray(inputs["b1_utt"], np.float32)
                  + np.asarray(inputs["b2_utt"], np.float32))

    cbias = np.asarray(inputs["ctx_bias"], np.float32)
    dbias = np.asarray(inputs["dec_bias"], np.float32)

    def gate_bias_row(bias2):
        return np.concatenate([
            bias2[0, :D] + bias2[1, :D],
            bias2[0, D:2 * D] + bias2[1, D:2 * D],
            2.0 * bias2[0, 2 * D:],
        ]).reshape(1, G3)

    cbx = _bf(gate_bias_row(cbias))
    dbx = _bf(gate_bias_row(dbias))
    cb1h = _bf(cbias[1, 2 * D:].reshape(1, D))
    db1h = _bf(dbias[1, 2 * D:].reshape(1, D))

    ones_b = _bf(np.ones((1, BL * T), np.float32))

    enc_r = enc.reshape(B, R, D)

    in_maps = []
    for core in range(NCORES):
        sl = slice(core * BL, (core + 1) * BL)
        enc_t = np.ascontiguousarray(
            enc_r[sl].transpose(0, 2, 1)
            .reshape(BL, C, 128, R)
            .transpose(0, 2, 1, 3))
        hid_c = hidden[sl]
        mask_t = np.ascontiguousarray(
            np.broadcast_to(-0.5 * maskf[sl].T[None, :, :], (128, T, BL)))
        in_maps.append({
            "enc_t": _f8(enc_t),
            "hidT_f": _f32(tmajor(hid_c)),
            "hidT_b": _bf(tmajor(hid_c)),
            "hidT_8": _f8(tmajor(hid_c)),
            "xembT_8": _f8(tmajor(x_emb[sl])),
            "w1w": w1w, "w2w": w2w, "vw_rep": vw,
            "w1u": w1u, "w2u": w2u, "vu_rep": vu,
            "ctxk": ctxk, "ctxrk": ctxrk,
            "deckA": deckA, "deckB": deckB, "decrk": decrk,
            "qb_w": qb_w, "qb_u": qb_u,
            "cbx_row": cbx, "cb1h_b": cb1h,
            "dbx_row": dbx, "db1h_b": db1h,
            "mask_t": _f32(mask_t),
            "ones_b": ones_b,
        })
    return in_maps


def run(inputs):
    nc = _get_nc()
    in_maps = prepare_in_maps(inputs)
    res = run_bass_kernel_spmd(nc, in_maps, list(range(NCORES)))
    # out per core: [128, C, BL] feature-major; host transposes to [BL, D]
    parts = []
    for c in range(NCORES):
        o = np.asarray(res.results[c]["out"])           # [128, C, BL]
        parts.append(o.transpose(2, 1, 0).reshape(BL, D))
    out = np.concatenate(parts, axis=0)
    return np.ascontiguousarray(out.astype(np.float32)), res


def kernel(**inputs):
    out, _ = run(inputs)
    return out, out


# revision 35
# speedup vs baseline: 1.7574x; 1.0072x over previous
"""Trainium2 Bass kernel for nn_Decoder_55688545960558 (v2, fp8).

Hierarchical-attention GRU decoder step, data-parallel over batch
(64 -> 8 per core), no collectives.

Key differences vs v1:
- All attention weights, enc, and the context-GRU weights in fp8e4
  (validated: rel_err ~2e-3); decoder GRU's recurrent kernel stays bf16.
  fp8 matmuls use DoubleRow perf mode (2 k-chunks per instruction).
- Word-attention softmax: exp is NOT normalized; the weighted sums are
  scaled by 1/Z at the end (saves a [128,500] DVE pass per batch).
- The weighted-sum multiplies are split DVE/GpSimd; reduce on DVE.
- All gate biases enter PSUM via rank-1 ones-matmuls; PSUM->SBUF moves
  are single wide ops (no per-chunk scalar adds).
- Output stays feature-major [128, C, BL]; the host transposes.
"""

from contextlib import ExitStack

import numpy as np
import ml_dtypes

import concourse.bass as bass
import concourse.mybir as mybir
import concourse.tile as tile
from concourse import bacc
from concourse.bass_utils import run_bass_kernel_spmd

F32 = mybir.dt.float32
BF16 = mybir.dt.bfloat16
FP8 = mybir.dt.float8e4
AF = mybir.ActivationFunctionType
OP = mybir.AluOpType
AX = mybir.AxisListType
DR = mybir.MatmulPerfMode.DoubleRow

NCORES = 8
B = 64
BL = B // NCORES  # 8
T = 10
S = 50
R = T * S         # 500
D = 1024
U = 1024
C = D // 128      # 8
CP = C // 2       # 4 k-pairs for DoubleRow
G3 = 3 * D        # 3072

DEBUG = False


def _bcast_mid(ap, n):
    """Insert a 0-stride broadcast dim of size n as dim 1 (after partitions)."""
    return bass.AP(tensor=ap.tensor, offset=ap.offset,
                   ap=[ap.ap[0], [0, n]] + list(ap.ap[1:]))


def _bcast_last(ap, n):
    return bass.AP(tensor=ap.tensor, offset=ap.offset,
                   ap=list(ap.ap) + [[0, n]])


def build():
    nc = bacc.Bacc("TRN2", target_bir_lowering=False, debug=False,
                   num_devices=NCORES)

    def din(name, shape, dt):
        return nc.dram_tensor(name, list(shape), dt, kind="ExternalInput").ap()

    ins = {}
    ins["enc"] = din("enc_t", [BL, 128, C, R], FP8)
    ins["hidT_f"] = din("hidT_f", [128, C, BL], F32)
    ins["hidT_b"] = din("hidT_b", [128, C, BL], BF16)
    ins["hidT_8"] = din("hidT_8", [128, C, BL], FP8)
    ins["xembT_8"] = din("xembT_8", [128, C, BL], FP8)
    ins["w1w"] = din("w1w", [128, C, U], FP8)
    ins["w2w"] = din("w2w", [128, C, U], FP8)
    ins["vw"] = din("vw_rep", [128, C, 128], FP8)
    ins["w1u"] = din("w1u", [128, C, U], FP8)
    ins["w2u"] = din("w2u", [128, C, U], FP8)
    ins["vu"] = din("vu_rep", [128, C, 128], FP8)
    ins["ctxk"] = din("ctxk", [128, C, G3], FP8)
    ins["ctxrk"] = din("ctxrk", [128, C, G3], FP8)
    ins["deckA"] = din("deckA", [128, C, G3], FP8)
    ins["deckB"] = din("deckB", [128, C, G3], FP8)
    ins["decrk"] = din("decrk", [C, 128, G3], BF16)
    ins["qb_w"] = din("qb_w", [128, C], F32)
    ins["qb_u"] = din("qb_u", [128, C], F32)
    ins["cbx_row"] = din("cbx_row", [1, G3], BF16)
    ins["cb1h"] = din("cb1h_b", [1, D], BF16)
    ins["dbx_row"] = din("dbx_row", [1, G3], BF16)
    ins["db1h"] = din("db1h_b", [1, D], BF16)
    ins["mask"] = din("mask_t", [128, T, BL], F32)   # pre-scaled by -0.5
    ins["ones"] = din("ones_b", [1, BL * T], BF16)

    ins["out"] = nc.dram_tensor("out", [128, C, BL], F32,
                                kind="ExternalOutput").ap()
    dbg = {}
    if DEBUG:
        dbg["q"] = nc.dram_tensor("dbg_q", [128, C, BL], F32,
                                  kind="ExternalOutput").ap()
        dbg["ctx"] = nc.dram_tensor("dbg_ctx", [128, C, BL, T], F32,
                                    kind="ExternalOutput").ap()
        dbg["seq"] = nc.dram_tensor("dbg_seq", [128, C, BL, T], F32,
                                    kind="ExternalOutput").ap()
        dbg["ctxv"] = nc.dram_tensor("dbg_ctxv", [128, C, BL], F32,
                                     kind="ExternalOutput").ap()
        dbg["hmd"] = nc.dram_tensor("dbg_hmd", [128, 3, C, BL], F32,
                                    kind="ExternalOutput").ap()
        dbg["xmd"] = nc.dram_tensor("dbg_xmd", [128, 3, C, BL], F32,
                                    kind="ExternalOutput").ap()
    ins["dbg"] = dbg

    with nc.allow_low_precision(reason="bf16/fp8 activations by design"):
        with tile.TileContext(nc) as tc:
            _emit(nc, tc, ins)
    nc.compile()
    return nc


def _emit(nc, tc, ins):
    dbg = ins["dbg"]
    es = ExitStack()

    pers = es.enter_context(tc.tile_pool(name="pers", bufs=1))
    wsA = es.enter_context(tc.tile_pool(name="wsA", bufs=1))    # w1w -> w1u
    wsB = es.enter_context(tc.tile_pool(name="wsB", bufs=1))    # w2w -> w2u
    gruw = es.enter_context(tc.tile_pool(name="gruw", bufs=1))  # ctxk/ctxrk
    decw = es.enter_context(tc.tile_pool(name="decw", bufs=1))  # deckA/deckB
    encp = es.enter_context(tc.tile_pool(name="encp", bufs=3))
    drkp = es.enter_context(tc.tile_pool(name="drkp", bufs=4))
    thp = es.enter_context(tc.tile_pool(name="thp", bufs=1))
    ep = es.enter_context(tc.tile_pool(name="ep", bufs=2))
    prp = es.enter_context(tc.tile_pool(name="prp", bufs=3))
    s1small = es.enter_context(tc.tile_pool(name="s1small", bufs=2))
    gtmp = es.enter_context(tc.tile_pool(name="gtmp", bufs=2))
    hstate = es.enter_context(tc.tile_pool(name="hstate", bufs=2))
    s34 = es.enter_context(tc.tile_pool(name="s34", bufs=2))

    def ld(pool, dram_ap, shape, dt, name, chunked=False):
        t = pool.tile(list(shape), dt, tag=name, name=name)
        if chunked:
            for k in range(shape[1]):
                nc.sync.dma_start(out=t[:, k], in_=dram_ap[:, k])
        else:
            nc.sync.dma_start(out=t[:], in_=dram_ap)
        return t

    # ---------------- DMA: critical-path order on the sync queue ----------
    w1w_s = ld(wsA, ins["w1w"], [128, C, U], FP8, "wA")
    enc_tiles = [ld(encp, ins["enc"][0], [128, C, R], FP8, "enc")]
    hidT_8 = ld(pers, ins["hidT_8"], [128, C, BL], FP8, "hidT_8")
    qbw_s = ld(pers, ins["qb_w"], [128, C], F32, "qbw")
    w2w_s = ld(wsB, ins["w2w"], [128, C, U], FP8, "wB")
    vw_s = ld(pers, ins["vw"], [128, C, 128], FP8, "vw")
    for b in range(1, BL):
        enc_tiles.append(ld(encp, ins["enc"][b], [128, C, R], FP8, "enc"))
    hidT_f = ld(pers, ins["hidT_f"], [128, C, BL], F32, "hidT_f")
    hidT_b = ld(pers, ins["hidT_b"], [128, C, BL], BF16, "hidT_b")
    xembT_8 = ld(pers, ins["xembT_8"], [128, C, BL], FP8, "xembT_8")
    qbu_s = ld(pers, ins["qb_u"], [128, C], F32, "qbu")
    mask_s = ld(pers, ins["mask"], [128, T, BL], F32, "mask")
    ones_s = ld(pers, ins["ones"], [1, BL * T], BF16, "ones")
    cbx_s = ld(pers, ins["cbx_row"], [1, G3], BF16, "cbx")
    cb1h_s = ld(pers, ins["cb1h"], [1, D], BF16, "cb1h")
    dbx_s = ld(pers, ins["dbx_row"], [1, G3], BF16, "dbx")
    db1h_s = ld(pers, ins["db1h"], [1, D], BF16, "db1h")
    ctxk_s = ld(gruw, ins["ctxk"], [128, C, G3], FP8, "ctxk")
    ctxrk_s = ld(gruw, ins["ctxrk"], [128, C, G3], FP8, "ctxrk")
    w1u_s = ld(wsA, ins["w1u"], [128, C, U], FP8, "wA")
    w2u_s = ld(wsB, ins["w2u"], [128, C, U], FP8, "wB")
    vu_s = ld(pers, ins["vu"], [128, C, 128], FP8, "vu")
    # decrk in 4 pair-DMAs so hm_dec can stream during the GRU scan
    decrk_tiles = []
    for j in range(C // 2):
        dk = drkp.tile([128, 2, G3], BF16, tag="drk", name=f"decrk{j}")
        nc.sync.dma_start(
            out=dk[:],
            in_=ins["decrk"][2 * j:2 * j + 2].rearrange("c p g -> p c g"))
        decrk_tiles.append(dk)
    # deckA rotates into deckB's slot (deckB is consumed by the xmdB
    # precompute before deckA's transfer may land)
    deckB_s = decw.tile([128, C, G3], FP8, tag="deck", name="deckB")
    nc.sync.dma_start(out=deckB_s[:], in_=ins["deckB"])
    deckA_s = decw.tile([128, C, G3], FP8, tag="deck", name="deckA")
    nc.sync.dma_start(out=deckA_s[:], in_=ins["deckA"])

    # cross-stage activations
    qsb = pers.tile([128, C, BL], F32, tag="qsb")
    qu_s = pers.tile([128, C, BL], F32, tag="qu")
    ctx8 = pers.tile([128, C, BL, T], FP8, tag="ctx8")
    seq8 = pers.tile([128, C, BL, T], FP8, tag="seq8")
    xg = [pers.tile([128, C, BL, T], FP8, tag=f"xg{g}", name=f"xg{g}")
          for g in range(3)]
    hmd_sb = pers.tile([128, 3, C, BL], F32, tag="hmd_sb")
    bh_sb = pers.tile([128, 3, C, BL], F32, tag="bh_sb")
    ctxv8 = pers.tile([128, C, BL], FP8, tag="ctxv8")

    # =================== STAGE 1: word attention ===================
    with tc.tile_pool(name="ps_score", bufs=5, space="PSUM") as p_score, \
         tc.tile_pool(name="pq", bufs=1, space="PSUM") as pq:
        def score_group(enc_b, m):
            ps = p_score.tile([128, R], F32, tag="ps")
            for kp in range(CP):
                nc.tensor.matmul(out=ps[:],
                                 lhsT=w1w_s[:, 2 * kp:2 * kp + 2,
                                            m * 128:(m + 1) * 128],
                                 rhs=enc_b[:, 2 * kp:2 * kp + 2],
                                 start=(kp == 0), stop=(kp == CP - 1),
                                 perf_mode=DR)
            return ps

        def q_matmuls(w_s, qb, out_sb):
            p_q = pq.tile([128, C, BL], F32, tag="pq")
            for mm in range(C):
                for kp in range(CP):
                    nc.tensor.matmul(out=p_q[:, mm],
                                     lhsT=w_s[:, 2 * kp:2 * kp + 2,
                                              mm * 128:(mm + 1) * 128],
                                     rhs=hidT_8[:, 2 * kp:2 * kp + 2],
                                     start=(kp == 0), stop=(kp == CP - 1),
                                     perf_mode=DR)
            for mm in range(C):
                nc.vector.tensor_scalar_add(out=out_sb[:, mm], in0=p_q[:, mm],
                                            scalar1=qb[:, mm:mm + 1])

        pending = None   # (pr, rc, b) of the previous batch

        def flush_pending():
            # reduce+scale for batch b-1, deferred so the in-order DVE queue
            # fills the wait on the Pool multiply with batch-b work
            nonlocal pending
            if pending is None:
                return
            pr_p, rc_p, b_p = pending
            red = s1small.tile([128, C, T], F32, tag="red")
            nc.vector.reduce_sum(out=red[:], in_=pr_p[:], axis=AX.X)
            nc.vector.tensor_tensor(out=ctx8[:, :, b_p, :], in0=red[:],
                                    in1=_bcast_mid(rc_p[:], C), op=OP.mult)
            pending = None

        for b in range(BL):
            enc_b = enc_tiles[b]
            th = thp.tile([128, C, R], FP8, tag="th")
            if b == 0:
                pss = [score_group(enc_b, m) for m in range(4)]
                q_matmuls(w2w_s, qbw_s, qsb)
                if DEBUG:
                    nc.sync.dma_start(out=dbg["q"], in_=qsb[:])
                for m in range(4):
                    nc.scalar.activation(out=th[:, m], in_=pss[m][:],
                                         func=AF.Tanh,
                                         bias=qsb[:, m, b:b + 1])
                for m in range(4, C):
                    ps = score_group(enc_b, m)
                    nc.scalar.activation(out=th[:, m], in_=ps[:],
                                         func=AF.Tanh,
                                         bias=qsb[:, m, b:b + 1])
            else:
                for m in range(C):
                    ps = score_group(enc_b, m)
                    nc.scalar.activation(out=th[:, m], in_=ps[:],
                                         func=AF.Tanh,
                                         bias=qsb[:, m, b:b + 1])
            # V matmul (replicated scores on all partitions)
            psc = p_score.tile([128, R], F32, tag="ps")
            for cp in range(CP):
                nc.tensor.matmul(out=psc[:], lhsT=vw_s[:, 2 * cp:2 * cp + 2],
                                 rhs=th[:, 2 * cp:2 * cp + 2],
                                 start=(cp == 0), stop=(cp == CP - 1),
                                 perf_mode=DR)
            e = s1small.tile([128, T, S], BF16, tag="e")
            nc.scalar.activation(
                out=e[:], in_=psc[:].rearrange("p (t s) -> p t s", s=S),
                func=AF.Exp)
            # unnormalized weighted sum: pr = enc * e  (DVE 2 chunks, Pool 6)
            pr = prp.tile([128, C, T, S], FP8, tag="pr")
            encv = enc_b[:].rearrange("p c (t s) -> p c t s", s=S)
            nc.vector.tensor_tensor(out=pr[:, 0:2], in0=encv[:, 0:2],
                                    in1=_bcast_mid(e[:], 2), op=OP.mult)
            nc.gpsimd.tensor_tensor(out=pr[:, 2:8], in0=encv[:, 2:8],
                                    in1=_bcast_mid(e[:], 6), op=OP.mult)
            rs = s1small.tile([128, T], F32, tag="rs")
            nc.vector.reduce_sum(out=rs[:], in_=e[:], axis=AX.X)
            rc = s1small.tile([128, T], F32, tag="rc")
            nc.vector.reciprocal(out=rc[:], in_=rs[:])
            flush_pending()
            pending = (pr, rc, b)
        flush_pending()
        # utt query after the batch loop: w2u's DMA lands late in the
        # stream, and the in-order PE queue must not stall stage 1 on it
        q_matmuls(w2u_s, qbu_s, qu_s)
    if DEBUG:
        dbg_ctx = pers.tile([128, C, BL, T], F32, tag="dbg_ctx")
        nc.vector.tensor_copy(out=dbg_ctx[:], in_=ctx8[:])
        nc.sync.dma_start(out=dbg["ctx"], in_=dbg_ctx[:])

    # =================== STAGE 2: context GRU ===================
    ctx8v = ctx8[:].rearrange("p c b t -> p c (b t)")
    with tc.tile_pool(name="ps_xm", bufs=3, space="PSUM") as ps_xm:
        for g in (2,):   # z/r xm fuse into the per-step phm groups instead
            for half in range(2):
                pxm = ps_xm.tile([128, 4, BL, T], F32, tag="pxm")
                for cc in range(4):
                    c = half * 4 + cc
                    col0 = g * D + c * 128
                    for kp in range(CP):
                        nc.tensor.matmul(
                            out=pxm[:, cc],
                            lhsT=ctxk_s[:, 2 * kp:2 * kp + 2, col0:col0 + 128],
                            rhs=ctx8v[:, 2 * kp:2 * kp + 2],
                            start=(kp == 0), stop=False, perf_mode=DR)
                    # bias as rank-1 ones-matmul closes the group
                    nc.tensor.matmul(out=pxm[:, cc],
                                     lhsT=cbx_s[:, col0:col0 + 128],
                                     rhs=ones_s[:], start=False, stop=True)
                nc.vector.tensor_copy(out=xg[g][:, half * 4:half * 4 + 4],
                                      in_=pxm[:])

    h_f = None
    ps_hmd = es.enter_context(tc.tile_pool(name="ps_hmd", bufs=2, space="PSUM"))
    ps_xmd = es.enter_context(tc.tile_pool(name="ps_xmd", bufs=1, space="PSUM"))
    xmdB_sb = pers.tile([128, 3, C, BL], F32, tag="xmdB_sb")

    with tc.tile_pool(name="ps_hm", bufs=2, space="PSUM") as ps_hm:
        for t in range(T):
            phm = ps_hm.tile([128, 3, C, BL], F32, tag="phm")
            for g in (1, 0, 2):   # r first: it gates the candidate chain
                for c in range(C):
                    col0 = g * D + c * 128
                    if t > 0:
                        for kp in range(CP):
                            nc.tensor.matmul(
                                out=phm[:, g, c],
                                lhsT=ctxrk_s[:, 2 * kp:2 * kp + 2,
                                             col0:col0 + 128],
                                rhs=seq8[:, 2 * kp:2 * kp + 2, :, t - 1],
                                start=(kp == 0), stop=False,
                                perf_mode=DR)
                    if g != 2:
                        # z/r: xm for this turn + bias fused into the group
                        for kp in range(CP):
                            nc.tensor.matmul(
                                out=phm[:, g, c],
                                lhsT=ctxk_s[:, 2 * kp:2 * kp + 2,
                                            col0:col0 + 128],
                                rhs=ctx8[:, 2 * kp:2 * kp + 2, :, t],
                                start=(t == 0 and kp == 0), stop=False,
                                perf_mode=DR)
                        nc.tensor.matmul(out=phm[:, g, c],
                                         lhsT=cbx_s[:, col0:col0 + 128],
                                         rhs=ones_s[:, :BL], start=False,
                                         stop=True)
                    else:
                        nc.tensor.matmul(out=phm[:, g, c],
                                         lhsT=cb1h_s[:, c * 128:(c + 1) * 128],
                                         rhs=ones_s[:, :BL], start=(t == 0),
                                         stop=True)
            if t == 3:
                # emb-half of the decoder input kernel in a GRU PE gap
                # (deckB has landed; copying to SBUF frees its slot so the
                # in-order DMA queue can start deckA's transfer)
                pxB = ps_xmd.tile([128, 3, C, BL], F32, tag="pxB")
                for g in range(3):
                    for c in range(C):
                        col0 = g * D + c * 128
                        for kp in range(CP):
                            nc.tensor.matmul(
                                out=pxB[:, g, c],
                                lhsT=deckB_s[:, 2 * kp:2 * kp + 2,
                                             col0:col0 + 128],
                                rhs=xembT_8[:, 2 * kp:2 * kp + 2],
                                start=(kp == 0), stop=False, perf_mode=DR)
                        nc.tensor.matmul(out=pxB[:, g, c],
                                         lhsT=dbx_s[:, col0:col0 + 128],
                                         rhs=ones_s[:, :BL], start=False,
                                         stop=True)
            if t == 4:
                nc.vector.tensor_copy(out=xmdB_sb[:], in_=pxB[:])
            # sigmoid(x) == (tanh(x/2)+1)/2; affine parts folded on host
            mask_bc = _bcast_mid(mask_s[:, t, :], C)
            tz = gtmp.tile([128, C, BL], F32, tag="tz")
            tr = gtmp.tile([128, C, BL], F32, tag="tr")
            nc.scalar.activation(out=tr[:], in_=phm[:, 1], func=AF.Tanh,
                                 scale=0.5)
            nc.scalar.activation(out=tz[:], in_=phm[:, 0], func=AF.Tanh,
                                 scale=0.5)
            # rhh = (tanh_r + 1) * hh == 2*r*hh; xg-h host-doubled
            # rhh = (tanh_r + 1) * hh; at t==0 phm[2] is the h-bias only
            rhh = gtmp.tile([128, C, BL], F32, tag="rhh")
            nc.vector.scalar_tensor_tensor(out=rhh[:], in0=tr[:],
                                           scalar=1.0, in1=phm[:, 2],
                                           op0=OP.add, op1=OP.mult)
            cin = gtmp.tile([128, C, BL], F32, tag="cin")
            nc.vector.tensor_tensor(out=cin[:], in0=xg[2][:, :, :, t],
                                    in1=rhh[:], op=OP.add)
            # zcm = (1-z)*mask == (tanh_z - 1) * (-0.5*mask)
            zcm = gtmp.tile([128, C, BL], F32, tag="zcm")
            nc.vector.scalar_tensor_tensor(out=zcm[:], in0=tz[:], scalar=-1.0,
                                           in1=mask_bc, op0=OP.add,
                                           op1=OP.mult)
            h_f2 = hstate.tile([128, C, BL], F32, tag="h_f")
            if t > 0:
                hz1 = gtmp.tile([128, C, BL], F32, tag="hz1")
                nc.vector.tensor_tensor(out=hz1[:], in0=h_f[:], in1=zcm[:],
                                        op=OP.mult)
                hm1 = gtmp.tile([128, C, BL], F32, tag="hm1")
                nc.vector.tensor_tensor(out=hm1[:], in0=h_f[:], in1=hz1[:],
                                        op=OP.subtract)
            cand = gtmp.tile([128, C, BL], F32, tag="cand")
            nc.scalar.activation(out=cand[:], in_=cin[:], func=AF.Tanh,
                                 scale=0.5)
            if t == 0:
                nc.vector.tensor_tensor(out=seq8[:, :, :, 0], in0=cand[:],
                                        in1=zcm[:], op=OP.mult)
                nc.vector.tensor_tensor(out=h_f2[:], in0=cand[:], in1=zcm[:],
                                        op=OP.mult)
            else:
                t2 = gtmp.tile([128, C, BL], F32, tag="t2")
                nc.vector.tensor_tensor(out=t2[:], in0=cand[:], in1=zcm[:],
                                        op=OP.mult)
                nc.vector.tensor_tensor(out=seq8[:, :, :, t], in0=hm1[:],
                                        in1=t2[:], op=OP.add)
                nc.vector.tensor_tensor(out=h_f2[:], in0=hm1[:], in1=t2[:],
                                        op=OP.add)
            h_f = h_f2
    if DEBUG:
        dbg_seq = pers.tile([128, C, BL, T], F32, tag="dbg_seq")
        nc.vector.tensor_copy(out=dbg_seq[:], in_=seq8[:])
        nc.sync.dma_start(out=dbg["seq"], in_=dbg_seq[:])

    # =================== STAGE 3: utterance attention ===================
    seq8v = seq8[:].rearrange("p c b t -> p c (b t)")
    with tc.tile_pool(name="ps_su", bufs=2, space="PSUM") as ps_su, \
         tc.tile_pool(name="ps_scu", bufs=1, space="PSUM") as ps_scu, \
         tc.tile_pool(name="s3tmp", bufs=2) as s3tmp:
        su8 = s3tmp.tile([128, C, BL, T], FP8, tag="su8")
        for half in range(2):
            psu = ps_su.tile([128, 4, BL, T], F32, tag="psu")
            for mm in range(4):
                m = half * 4 + mm
                for kp in range(CP):
                    nc.tensor.matmul(
                        out=psu[:, mm],
                        lhsT=w1u_s[:, 2 * kp:2 * kp + 2,
                                   m * 128:(m + 1) * 128],
                        rhs=seq8v[:, 2 * kp:2 * kp + 2],
                        start=(kp == 0), stop=(kp == CP - 1), perf_mode=DR)
            qn = s3tmp.tile([128, 4, BL, T], F32, tag="qn")
            nc.vector.tensor_tensor(
                out=qn[:], in0=psu[:],
                in1=_bcast_last(qu_s[:, half * 4:half * 4 + 4], T), op=OP.add)
            nc.scalar.activation(out=su8[:, half * 4:half * 4 + 4],
                                 in_=qn[:], func=AF.Tanh)
        # hm_dec = hidden @ dec_rec_kernel on PE under the stage-3
        # softmax's DVE/ACT shadow (decrk pairs have landed by now)
        for k in range(C):
            dk = decrk_tiles[k // 2][:, k % 2]
            phmd_k = ps_hmd.tile([128, 3, C, BL], F32, tag="phmd_k")
            for g in range(3):
                for c in range(C):
                    col0 = g * D + c * 128
                    nc.tensor.matmul(out=phmd_k[:, g, c],
                                     lhsT=dk[:, col0:col0 + 128],
                                     rhs=hidT_b[:, k], start=True,
                                     stop=(k != C - 1 or g != 2))
                    if k == C - 1 and g == 2:
                        nc.tensor.matmul(
                            out=phmd_k[:, g, c],
                            lhsT=db1h_s[:, c * 128:(c + 1) * 128],
                            rhs=ones_s[:, :BL], start=False, stop=True)
            if k == 0:
                nc.vector.tensor_copy(out=hmd_sb[:], in_=phmd_k[:])
            else:
                nc.vector.tensor_tensor(out=hmd_sb[:], in0=hmd_sb[:],
                                        in1=phmd_k[:], op=OP.add)
        pscu = ps_scu.tile([128, BL, T], F32)
        for cp in range(CP):
            nc.tensor.matmul(out=pscu[:], lhsT=vu_s[:, 2 * cp:2 * cp + 2],
                             rhs=su8[:, 2 * cp:2 * cp + 2],
                             start=(cp == 0), stop=(cp == CP - 1),
                             perf_mode=DR)
        eu = s3tmp.tile([128, BL, T], BF16, tag="eu")
        nc.scalar.activation(out=eu[:], in_=pscu[:], func=AF.Exp)
        rsu = s3tmp.tile([128, BL], F32, tag="rsu")
        nc.vector.reduce_sum(out=rsu[:], in_=eu[:], axis=AX.X)
        rcu = s3tmp.tile([128, BL], F32, tag="rcu")
        nc.vector.reciprocal(out=rcu[:], in_=rsu[:])
        pru = s3tmp.tile([128, C, BL, T], BF16, tag="pru")
        nc.vector.tensor_tensor(out=pru[:], in0=seq8[:],
                                in1=_bcast_mid(eu[:], C), op=OP.mult)
        redu = s3tmp.tile([128, C, BL], F32, tag="redu")
        nc.vector.reduce_sum(out=redu[:], in_=pru[:], axis=AX.X)
        nc.vector.tensor_tensor(out=ctxv8[:], in0=redu[:],
                                in1=_bcast_mid(rcu[:], C), op=OP.mult)
    if DEBUG:
        dbg_cv = pers.tile([128, C, BL], F32, tag="dbg_cv")
        nc.vector.tensor_copy(out=dbg_cv[:], in_=ctxv8[:])
        nc.sync.dma_start(out=dbg["ctxv"], in_=dbg_cv[:])

    # =================== STAGE 4: decoder GRU step ===================
    with tc.tile_pool(name="s4tmp", bufs=1) as s4tmp:
        nc.vector.tensor_tensor(out=bh_sb[:], in0=hmd_sb[:], in1=xmdB_sb[:],
                                op=OP.add)
        if DEBUG:
            nc.sync.dma_start(out=dbg["hmd"], in_=hmd_sb[:])

        # ctxv-half (deckA)
        pxA = ps_xmd.tile([128, 3, C, BL], F32, tag="pxA")
        for g in range(3):
            for c in range(C):
                col0 = g * D + c * 128
                for kp in range(CP):
                    nc.tensor.matmul(
                        out=pxA[:, g, c],
                        lhsT=deckA_s[:, 2 * kp:2 * kp + 2, col0:col0 + 128],
                        rhs=ctxv8[:, 2 * kp:2 * kp + 2],
                        start=(kp == 0), stop=(kp == CP - 1), perf_mode=DR)
        if DEBUG:
            dbg_xm = s4tmp.tile([128, 3, C, BL], F32, tag="dbg_xm")
            nc.vector.tensor_tensor(out=dbg_xm[:], in0=xmdB_sb[:],
                                    in1=pxA[:], op=OP.add)
            nc.sync.dma_start(out=dbg["xmd"], in_=dbg_xm[:])

        tz = s4tmp.tile([128, C, BL], F32, tag="tz4")
        tr = s4tmp.tile([128, C, BL], F32, tag="tr4")
        rin = s4tmp.tile([128, C, BL], F32, tag="rin4")
        nc.vector.tensor_tensor(out=rin[:], in0=pxA[:, 1], in1=bh_sb[:, 1],
                                op=OP.add)
        nc.scalar.activation(out=tr[:], in_=rin[:], func=AF.Tanh, scale=0.5)
        zin = s4tmp.tile([128, C, BL], F32, tag="zin4")
        nc.vector.tensor_tensor(out=zin[:], in0=pxA[:, 0], in1=bh_sb[:, 0],
                                op=OP.add)
        nc.scalar.activation(out=tz[:], in_=zin[:], func=AF.Tanh, scale=0.5)
        # candidate: cin/2 = xh + r*hh with xh = xA_h + xB_h + b0_h (host-
        # doubled cols/bias), hh = hmd_h + b1_h (plain). bh[2] mixes hmd_h
        # into the x-side, so use pxB[2] and hmd_sb[2] directly here.
        rhh = s4tmp.tile([128, C, BL], F32, tag="rhh4")
        nc.vector.scalar_tensor_tensor(out=rhh[:], in0=tr[:], scalar=1.0,
                                       in1=hmd_sb[:, 2], op0=OP.add,
                                       op1=OP.mult)
        xh = s4tmp.tile([128, C, BL], F32, tag="xh4")
        nc.vector.tensor_tensor(out=xh[:], in0=pxA[:, 2], in1=xmdB_sb[:, 2],
                                op=OP.add)
        cin = s4tmp.tile([128, C, BL], F32, tag="cin4")
        nc.vector.tensor_tensor(out=cin[:], in0=xh[:], in1=rhh[:], op=OP.add)
        cand = s4tmp.tile([128, C, BL], F32, tag="cand4")
        nc.scalar.activation(out=cand[:], in_=cin[:], func=AF.Tanh, scale=0.5)
        zcm = s4tmp.tile([128, C, BL], F32, tag="zcm4")
        nc.vector.tensor_scalar(out=zcm[:], in0=tz[:], scalar1=-1.0,
                                scalar2=-0.5, op0=OP.add, op1=OP.mult)
        d1 = s4tmp.tile([128, C, BL], F32, tag="d14")
        nc.vector.tensor_tensor(out=d1[:], in0=cand[:], in1=hidT_f[:],
                                op=OP.subtract)
        d2 = s4tmp.tile([128, C, BL], F32, tag="d24")
        nc.vector.tensor_tensor(out=d2[:], in0=d1[:], in1=zcm[:], op=OP.mult)
        stT = s4tmp.tile([128, C, BL], F32, tag="stT")
        nc.vector.tensor_tensor(out=stT[:], in0=hidT_f[:], in1=d2[:],
                                op=OP.add)
        nc.sync.dma_start(out=ins["out"], in_=stT[:])

    es.close()


# ---------------------------------------------------------------------------
# Host side
# ---------------------------------------------------------------------------

_NC_CACHE = {}


def _get_nc():
    key = ("prog_v2", DEBUG)
    if key not in _NC_CACHE:
        _NC_CACHE[key] = build()
    return _NC_CACHE[key]


def _f8(a):
    return np.ascontiguousarray(np.asarray(a, np.float32)
                                .astype(ml_dtypes.float8_e4m3fn))


def _bf(a):
    return np.ascontiguousarray(np.asarray(a, np.float32)
                                .astype(ml_dtypes.bfloat16))


def _f32(a):
    return np.ascontiguousarray(np.asarray(a, np.float32))


def _chunked_T(w):
    """[D_in, N] -> [128, D_in//128, N]: row-chunked per-k lhsT tiles."""
    d_in, n = w.shape
    return np.ascontiguousarray(w.reshape(d_in // 128, 128, n)
                                .transpose(1, 0, 2))


def prepare_in_maps(inputs):
    x = np.asarray(inputs["x"]).astype(np.int64).reshape(B)
    hidden = _f32(inputs["hidden"])
    enc = _f32(inputs["encoder_outputs"])          # [64, 10, 50, 1024]
    maskf = np.asarray(inputs["context_mask"]).astype(np.float32)
    emb = np.asarray(inputs["embed_table"])

    x_emb = emb[x].astype(np.float32)

    def tmajor(a2d):  # [B, D] -> [128, C, B]
        return np.ascontiguousarray(
            a2d.T.reshape(C, 128, a2d.shape[0]).transpose(1, 0, 2))

    def dbl_h(w):
        w = np.array(w, np.float32, copy=True)
        w[:, 2 * D:] *= 2.0
        return w

    w1w = _f8(_chunked_T(np.asarray(inputs["w1_word"], np.float32)))
    w2w = _f8(_chunked_T(np.asarray(inputs["w2_word"], np.float32)))
    w1u = _f8(_chunked_T(np.asarray(inputs["w1_utt"], np.float32)))
    w2u = _f8(_chunked_T(np.asarray(inputs["w2_utt"], np.float32)))
    ctxk = _f8(_chunked_T(dbl_h(np.asarray(inputs["ctx_kernel"], np.float32))))
    ctxrk = _f8(_chunked_T(np.asarray(inputs["ctx_rec_kernel"], np.float32)))
    deck_full = dbl_h(np.asarray(inputs["dec_kernel"], np.float32))
    deckA = _f8(_chunked_T(deck_full[:D]))
    deckB = _f8(_chunked_T(deck_full[D:]))
    decrk = _bf(np.asarray(inputs["dec_rec_kernel"], np.float32)
                .reshape(C, 128, G3))

    def vrep(v):
        vc = np.asarray(v, np.float32).reshape(C, 128).T
        return _f8(np.broadcast_to(vc[:, :, None], (128, C, 128)))

    vw = vrep(inputs["v_word"])
    vu = vrep(inputs["v_utt"])

    def mchunk(v):
        return _f32(np.asarray(v, np.float32).reshape(C, 128).T)

    qb_w = mchunk(np.asarray(inputs["b1_word"], np.float32)
                  + np.asarray(inputs["b2_word"], np.float32))
    qb_u = mchunk(np.asarray(inputs["b1_utt"], np.float32)
                  + np.asarray(inputs["b2_utt"], np.float32))

    cbias = np.asarray(inputs["ctx_bias"], np.float32)
    dbias = np.asarray(inputs["dec_bias"], np.float32)

    def gate_bias_row(bias2):
        return np.concatenate([
            bias2[0, :D] + bias2[1, :D],
            bias2[0, D:2 * D] + bias2[1, D:2 * D],
            2.0 * bias2[0, 2 * D:],
        ]).reshape(1, G3)

    cbx = _bf(gate_bias_row(cbias))
    dbx = _bf(gate_bias_row(dbias))
    cb1h = _bf(cbias[1, 2 * D:].reshape(1, D))
    db1h = _bf(dbias[1, 2 * D:].reshape(1, D))

    ones_b = _bf(np.ones((1, BL * T), np.float32))

    enc_r = enc.reshape(B, R, D)

    in_maps = []
    for core in range(NCORES):
        sl = slice(core * BL, (core + 1) * BL)
        enc_t = np.ascontiguousarray(
            enc_r[sl].transpose(0, 2, 1)
            .reshape(BL, C, 128, R)
            .transpose(0, 2, 1, 3))
        hid_c = hidden[sl]
        mask_t = np.ascontiguousarray(
            np.broadcast_to(-0.5 * maskf[sl].T[None, :, :], (128, T, BL)))
        in_maps.append({
            "enc_t": _f8(enc_t),
            "hidT_f": _f32(tmajor(hid_c)),
            "hidT_b": _bf(tmajor(hid_c)),
            "hidT_8": _f8(tmajor(hid_c)),
            "xembT_8": _f8(tmajor(x_emb[sl])),
            "w1w": w1w, "w2w": w2w, "vw_rep": vw,
            "w1u": w1u, "w2u": w2u, "vu_rep": vu,
            "ctxk": ctxk, "ctxrk": ctxrk,
            "deckA": deckA, "deckB": deckB, "decrk": decrk,
            "qb_w": qb_w, "qb_u": qb_u,
            "cbx_row": cbx, "cb1h_b": cb1h,
            "dbx_row": dbx, "db1h_b": db1h,
            "mask_t": _f32(mask_t),
            "ones_b": ones_b,
        })
    return in_maps


def run(inputs):
    nc = _get_nc()
    in_maps = prepare_in_maps(inputs)
    res = run_bass_kernel_spmd(nc, in_maps, list(range(NCORES)))
    # out per core: [128, C, BL] feature-major; host transposes to [BL, D]
    parts = []
    for c in range(NCORES):
        o = np.asarray(res.results[c]["out"])           # [128, C, BL]
        parts.append(o.transpose(2, 1, 0).reshape(BL, D))
    out = np.concatenate(parts, axis=0)
    return np.ascontiguousarray(out.astype(np.float32)), res


def kernel(**inputs):
    out, _ = run(inputs)
    return out, out


# revision 37
# speedup vs baseline: 1.8152x; 1.0329x over previous
"""Trainium2 Bass kernel for nn_Decoder_55688545960558 (v2, fp8).

Hierarchical-attention GRU decoder step, data-parallel over batch
(64 -> 8 per core), no collectives.

Key differences vs v1:
- All attention weights, enc, and the context-GRU weights in fp8e4
  (validated: rel_err ~2e-3); decoder GRU's recurrent kernel stays bf16.
  fp8 matmuls use DoubleRow perf mode (2 k-chunks per instruction).
- Word-attention softmax: exp is NOT normalized; the weighted sums are
  scaled by 1/Z at the end (saves a [128,500] DVE pass per batch).
- The weighted-sum multiplies are split DVE/GpSimd; reduce on DVE.
- All gate biases enter PSUM via rank-1 ones-matmuls; PSUM->SBUF moves
  are single wide ops (no per-chunk scalar adds).
- Output stays feature-major [128, C, BL]; the host transposes.
"""

from contextlib import ExitStack

import numpy as np
import ml_dtypes

import concourse.bass as bass
import concourse.mybir as mybir
import concourse.tile as tile
from concourse import bacc
from concourse.bass_utils import run_bass_kernel_spmd

F32 = mybir.dt.float32
BF16 = mybir.dt.bfloat16
FP8 = mybir.dt.float8e4
AF = mybir.ActivationFunctionType
OP = mybir.AluOpType
AX = mybir.AxisListType
DR = mybir.MatmulPerfMode.DoubleRow

NCORES = 8
B = 64
BL = B // NCORES  # 8
T = 10
S = 50
R = T * S         # 500
D = 1024
U = 1024
C = D // 128      # 8
CP = C // 2       # 4 k-pairs for DoubleRow
G3 = 3 * D        # 3072

DEBUG = False


def _bcast_mid(ap, n):
    """Insert a 0-stride broadcast dim of size n as dim 1 (after partitions)."""
    return bass.AP(tensor=ap.tensor, offset=ap.offset,
                   ap=[ap.ap[0], [0, n]] + list(ap.ap[1:]))


def _bcast_last(ap, n):
    return bass.AP(tensor=ap.tensor, offset=ap.offset,
                   ap=list(ap.ap) + [[0, n]])


def build():
    nc = bacc.Bacc("TRN2", target_bir_lowering=False, debug=False,
                   num_devices=NCORES)

    def din(name, shape, dt):
        return nc.dram_tensor(name, list(shape), dt, kind="ExternalInput").ap()

    ins = {}
    ins["enc"] = din("enc_t", [BL, 128, C, R], FP8)
    ins["hidT_f"] = din("hidT_f", [128, C, BL], F32)
    ins["hidT_b"] = din("hidT_b", [128, C, BL], BF16)
    ins["hidT_8"] = din("hidT_8", [128, C, BL], FP8)
    ins["xembT_8"] = din("xembT_8", [128, C, BL], FP8)
    ins["w1w"] = din("w1w", [128, C, U], FP8)
    ins["w2w"] = din("w2w", [128, C, U], FP8)
    ins["vw"] = din("vw_rep", [128, C, 128], FP8)
    ins["w1u"] = din("w1u", [128, C, U], FP8)
    ins["w2u"] = din("w2u", [128, C, U], FP8)
    ins["vu"] = din("vu_rep", [128, C, 128], FP8)
    ins["ctxk"] = din("ctxk", [128, C, G3], FP8)
    ins["ctxrk"] = din("ctxrk", [128, C, G3], FP8)
    ins["deckA"] = din("deckA", [128, C, G3], FP8)
    ins["deckB"] = din("deckB", [128, C, G3], FP8)
    ins["decrk"] = din("decrk", [C, 128, G3], BF16)
    ins["qb_w"] = din("qb_w", [128, C], F32)
    ins["qb_u"] = din("qb_u", [128, C], F32)
    ins["cbx_row"] = din("cbx_row", [1, G3], BF16)
    ins["cb1h"] = din("cb1h_b", [1, D], BF16)
    ins["dbx_row"] = din("dbx_row", [1, G3], BF16)
    ins["db1h"] = din("db1h_b", [1, D], BF16)
    ins["mask"] = din("mask_t", [128, T, BL], F32)   # pre-scaled by -0.5
    ins["ones"] = din("ones_b", [1, BL * T], BF16)

    ins["out"] = nc.dram_tensor("out", [128, C, BL], F32,
                                kind="ExternalOutput").ap()
    dbg = {}
    if DEBUG:
        dbg["q"] = nc.dram_tensor("dbg_q", [128, C, BL], F32,
                                  kind="ExternalOutput").ap()
        dbg["ctx"] = nc.dram_tensor("dbg_ctx", [128, C, BL, T], F32,
                                    kind="ExternalOutput").ap()
        dbg["seq"] = nc.dram_tensor("dbg_seq", [128, C, BL, T], F32,
                                    kind="ExternalOutput").ap()
        dbg["ctxv"] = nc.dram_tensor("dbg_ctxv", [128, C, BL], F32,
                                     kind="ExternalOutput").ap()
        dbg["hmd"] = nc.dram_tensor("dbg_hmd", [128, 3, C, BL], F32,
                                    kind="ExternalOutput").ap()
        dbg["xmd"] = nc.dram_tensor("dbg_xmd", [128, 3, C, BL], F32,
                                    kind="ExternalOutput").ap()
    ins["dbg"] = dbg

    with nc.allow_low_precision(reason="bf16/fp8 activations by design"):
        with tile.TileContext(nc) as tc:
            _emit(nc, tc, ins)
    nc.compile()
    return nc


def _emit(nc, tc, ins):
    dbg = ins["dbg"]
    es = ExitStack()

    pers = es.enter_context(tc.tile_pool(name="pers", bufs=1))
    wsA = es.enter_context(tc.tile_pool(name="wsA", bufs=1))    # w1w -> w1u
    wsB = es.enter_context(tc.tile_pool(name="wsB", bufs=1))    # w2w -> w2u
    gruw = es.enter_context(tc.tile_pool(name="gruw", bufs=1))  # ctxk/ctxrk
    decw = es.enter_context(tc.tile_pool(name="decw", bufs=1))  # deckA/deckB
    encp = es.enter_context(tc.tile_pool(name="encp", bufs=4))
    drkp = es.enter_context(tc.tile_pool(name="drkp", bufs=4))
    thp = es.enter_context(tc.tile_pool(name="thp", bufs=1))
    ep = es.enter_context(tc.tile_pool(name="ep", bufs=2))
    prp = es.enter_context(tc.tile_pool(name="prp", bufs=3))
    s1small = es.enter_context(tc.tile_pool(name="s1small", bufs=2))
    gtmp = es.enter_context(tc.tile_pool(name="gtmp", bufs=2))
    hstate = es.enter_context(tc.tile_pool(name="hstate", bufs=2))
    s34 = es.enter_context(tc.tile_pool(name="s34", bufs=2))

    def ld(pool, dram_ap, shape, dt, name, chunked=False):
        t = pool.tile(list(shape), dt, tag=name, name=name)
        if chunked:
            for k in range(shape[1]):
                nc.sync.dma_start(out=t[:, k], in_=dram_ap[:, k])
        else:
            nc.sync.dma_start(out=t[:], in_=dram_ap)
        return t

    # ---------------- DMA: critical-path order on the sync queue ----------
    w1w_s = ld(wsA, ins["w1w"], [128, C, U], FP8, "wA")
    enc_tiles = [ld(encp, ins["enc"][0], [128, C, R], FP8, "enc")]
    hidT_8 = ld(pers, ins["hidT_8"], [128, C, BL], FP8, "hidT_8")
    qbw_s = ld(pers, ins["qb_w"], [128, C], F32, "qbw")
    w2w_s = ld(wsB, ins["w2w"], [128, C, U], FP8, "wB")
    vw_s = ld(pers, ins["vw"], [128, C, 128], FP8, "vw")
    for b in range(1, BL):
        enc_tiles.append(ld(encp, ins["enc"][b], [128, C, R], FP8, "enc"))
    hidT_f = ld(pers, ins["hidT_f"], [128, C, BL], F32, "hidT_f")
    hidT_b = ld(pers, ins["hidT_b"], [128, C, BL], BF16, "hidT_b")
    xembT_8 = ld(pers, ins["xembT_8"], [128, C, BL], FP8, "xembT_8")
    qbu_s = ld(pers, ins["qb_u"], [128, C], F32, "qbu")
    mask_s = ld(pers, ins["mask"], [128, T, BL], F32, "mask")
    ones_s = ld(pers, ins["ones"], [1, BL * T], BF16, "ones")
    cbx_s = ld(pers, ins["cbx_row"], [1, G3], BF16, "cbx")
    cb1h_s = ld(pers, ins["cb1h"], [1, D], BF16, "cb1h")
    dbx_s = ld(pers, ins["dbx_row"], [1, G3], BF16, "dbx")
    db1h_s = ld(pers, ins["db1h"], [1, D], BF16, "db1h")
    ctxk_s = ld(gruw, ins["ctxk"], [128, C, G3], FP8, "ctxk")
    ctxrk_s = ld(gruw, ins["ctxrk"], [128, C, G3], FP8, "ctxrk")
    w1u_s = ld(wsA, ins["w1u"], [128, C, U], FP8, "wA")
    w2u_s = ld(wsB, ins["w2u"], [128, C, U], FP8, "wB")
    vu_s = ld(pers, ins["vu"], [128, C, 128], FP8, "vu")
    # decrk in 4 pair-DMAs so hm_dec can stream during the GRU scan
    decrk_tiles = []
    for j in range(C // 2):
        dk = drkp.tile([128, 2, G3], BF16, tag="drk", name=f"decrk{j}")
        nc.sync.dma_start(
            out=dk[:],
            in_=ins["decrk"][2 * j:2 * j + 2].rearrange("c p g -> p c g"))
        decrk_tiles.append(dk)
    # deckA rotates into deckB's slot (deckB is consumed by the xmdB
    # precompute before deckA's transfer may land)
    deckB_s = decw.tile([128, C, G3], FP8, tag="deck", name="deckB")
    nc.sync.dma_start(out=deckB_s[:], in_=ins["deckB"])
    deckA_s = decw.tile([128, C, G3], FP8, tag="deck", name="deckA")
    nc.sync.dma_start(out=deckA_s[:], in_=ins["deckA"])

    # cross-stage activations
    qsb = pers.tile([128, C, BL], F32, tag="qsb")
    qu_s = pers.tile([128, C, BL], F32, tag="qu")
    ctx8 = pers.tile([128, C, BL, T], FP8, tag="ctx8")
    seq8 = pers.tile([128, C, BL, T], FP8, tag="seq8")
    xg = [pers.tile([128, C, BL, T], FP8, tag=f"xg{g}", name=f"xg{g}")
          for g in range(3)]
    hmd_sb = pers.tile([128, 3, C, BL], F32, tag="hmd_sb")
    bh_sb = pers.tile([128, 3, C, BL], F32, tag="bh_sb")
    ctxv8 = pers.tile([128, C, BL], FP8, tag="ctxv8")

    # =================== STAGE 1: word attention ===================
    with tc.tile_pool(name="ps_score", bufs=5, space="PSUM") as p_score, \
         tc.tile_pool(name="pq", bufs=1, space="PSUM") as pq:
        def score_group(enc_b, m):
            ps = p_score.tile([128, R], F32, tag="ps")
            for kp in range(CP):
                nc.tensor.matmul(out=ps[:],
                                 lhsT=w1w_s[:, 2 * kp:2 * kp + 2,
                                            m * 128:(m + 1) * 128],
                                 rhs=enc_b[:, 2 * kp:2 * kp + 2],
                                 start=(kp == 0), stop=(kp == CP - 1),
                                 perf_mode=DR)
            return ps

        def q_matmuls(w_s, qb, out_sb):
            p_q = pq.tile([128, C, BL], F32, tag="pq")
            for mm in range(C):
                for kp in range(CP):
                    nc.tensor.matmul(out=p_q[:, mm],
                                     lhsT=w_s[:, 2 * kp:2 * kp + 2,
                                              mm * 128:(mm + 1) * 128],
                                     rhs=hidT_8[:, 2 * kp:2 * kp + 2],
                                     start=(kp == 0), stop=(kp == CP - 1),
                                     perf_mode=DR)
            for mm in range(C):
                nc.vector.tensor_scalar_add(out=out_sb[:, mm], in0=p_q[:, mm],
                                            scalar1=qb[:, mm:mm + 1])

        pending = None   # (pr, rc, b) of the previous batch

        def flush_pending():
            # reduce+scale for batch b-1, deferred so the in-order DVE queue
            # fills the wait on the Pool multiply with batch-b work
            nonlocal pending
            if pending is None:
                return
            pr_p, rc_p, b_p = pending
            red = s1small.tile([128, C, T], F32, tag="red")
            nc.vector.reduce_sum(out=red[:], in_=pr_p[:], axis=AX.X)
            nc.vector.tensor_tensor(out=ctx8[:, :, b_p, :], in0=red[:],
                                    in1=_bcast_mid(rc_p[:], C), op=OP.mult)
            pending = None

        for b in range(BL):
            enc_b = enc_tiles[b]
            th = thp.tile([128, C, R], FP8, tag="th")
            if b == 0:
                pss = [score_group(enc_b, m) for m in range(4)]
                q_matmuls(w2w_s, qbw_s, qsb)
                if DEBUG:
                    nc.sync.dma_start(out=dbg["q"], in_=qsb[:])
                for m in range(4):
                    nc.scalar.activation(out=th[:, m], in_=pss[m][:],
                                         func=AF.Tanh,
                                         bias=qsb[:, m, b:b + 1])
                for m in range(4, C):
                    ps = score_group(enc_b, m)
                    nc.scalar.activation(out=th[:, m], in_=ps[:],
                                         func=AF.Tanh,
                                         bias=qsb[:, m, b:b + 1])
            else:
                for m in range(C):
                    ps = score_group(enc_b, m)
                    nc.scalar.activation(out=th[:, m], in_=ps[:],
                                         func=AF.Tanh,
                                         bias=qsb[:, m, b:b + 1])
            # V matmul (replicated scores on all partitions)
            psc = p_score.tile([128, R], F32, tag="ps")
            for cp in range(CP):
                nc.tensor.matmul(out=psc[:], lhsT=vw_s[:, 2 * cp:2 * cp + 2],
                                 rhs=th[:, 2 * cp:2 * cp + 2],
                                 start=(cp == 0), stop=(cp == CP - 1),
                                 perf_mode=DR)
            e = s1small.tile([128, T, S], BF16, tag="e")
            nc.scalar.activation(
                out=e[:], in_=psc[:].rearrange("p (t s) -> p t s", s=S),
                func=AF.Exp)
            # unnormalized weighted sum: pr = enc * e  (DVE 2 chunks, Pool 6)
            pr = prp.tile([128, C, T, S], FP8, tag="pr")
            encv = enc_b[:].rearrange("p c (t s) -> p c t s", s=S)
            nc.vector.tensor_tensor(out=pr[:, 0:2], in0=encv[:, 0:2],
                                    in1=_bcast_mid(e[:], 2), op=OP.mult)
            nc.gpsimd.tensor_tensor(out=pr[:, 2:8], in0=encv[:, 2:8],
                                    in1=_bcast_mid(e[:], 6), op=OP.mult)
            rs = s1small.tile([128, T], F32, tag="rs")
            nc.vector.reduce_sum(out=rs[:], in_=e[:], axis=AX.X)
            rc = s1small.tile([128, T], F32, tag="rc")
            nc.vector.reciprocal(out=rc[:], in_=rs[:])
            flush_pending()
            pending = (pr, rc, b)
        flush_pending()
        # utt query after the batch loop: w2u's DMA lands late in the
        # stream, and the in-order PE queue must not stall stage 1 on it
        q_matmuls(w2u_s, qbu_s, qu_s)
    if DEBUG:
        dbg_ctx = pers.tile([128, C, BL, T], F32, tag="dbg_ctx")
        nc.vector.tensor_copy(out=dbg_ctx[:], in_=ctx8[:])
        nc.sync.dma_start(out=dbg["ctx"], in_=dbg_ctx[:])

    # =================== STAGE 2: context GRU ===================
    ctx8v = ctx8[:].rearrange("p c b t -> p c (b t)")
    with tc.tile_pool(name="ps_xm", bufs=3, space="PSUM") as ps_xm:
        for g in (2,):   # z/r xm fuse into the per-step phm groups instead
            for half in range(2):
                pxm = ps_xm.tile([128, 4, BL, T], F32, tag="pxm")
                for cc in range(4):
                    c = half * 4 + cc
                    col0 = g * D + c * 128
                    for kp in range(CP):
                        nc.tensor.matmul(
                            out=pxm[:, cc],
                            lhsT=ctxk_s[:, 2 * kp:2 * kp + 2, col0:col0 + 128],
                            rhs=ctx8v[:, 2 * kp:2 * kp + 2],
                            start=(kp == 0), stop=False, perf_mode=DR)
                    # bias as rank-1 ones-matmul closes the group
                    nc.tensor.matmul(out=pxm[:, cc],
                                     lhsT=cbx_s[:, col0:col0 + 128],
                                     rhs=ones_s[:], start=False, stop=True)
                nc.vector.tensor_copy(out=xg[g][:, half * 4:half * 4 + 4],
                                      in_=pxm[:])

    h_f = None
    ps_hmd = es.enter_context(tc.tile_pool(name="ps_hmd", bufs=2, space="PSUM"))
    ps_xmd = es.enter_context(tc.tile_pool(name="ps_xmd", bufs=1, space="PSUM"))
    xmdB_sb = pers.tile([128, 3, C, BL], F32, tag="xmdB_sb")

    with tc.tile_pool(name="ps_hm", bufs=3, space="PSUM") as ps_hm:
        for t in range(T):
            phm = ps_hm.tile([128, 3, C, BL], F32, tag="phm")
            for g in (1, 0, 2):   # r first: it gates the candidate chain
                for c in range(C):
                    col0 = g * D + c * 128
                    if t > 0:
                        for kp in range(CP):
                            nc.tensor.matmul(
                                out=phm[:, g, c],
                                lhsT=ctxrk_s[:, 2 * kp:2 * kp + 2,
                                             col0:col0 + 128],
                                rhs=seq8[:, 2 * kp:2 * kp + 2, :, t - 1],
                                start=(kp == 0), stop=False,
                                perf_mode=DR)
                    if g != 2:
                        # z/r: xm for this turn + bias fused into the group
                        for kp in range(CP):
                            nc.tensor.matmul(
                                out=phm[:, g, c],
                                lhsT=ctxk_s[:, 2 * kp:2 * kp + 2,
                                            col0:col0 + 128],
                                rhs=ctx8[:, 2 * kp:2 * kp + 2, :, t],
                                start=(t == 0 and kp == 0), stop=False,
                                perf_mode=DR)
                        nc.tensor.matmul(out=phm[:, g, c],
                                         lhsT=cbx_s[:, col0:col0 + 128],
                                         rhs=ones_s[:, :BL], start=False,
                                         stop=True)
                    else:
                        nc.tensor.matmul(out=phm[:, g, c],
                                         lhsT=cb1h_s[:, c * 128:(c + 1) * 128],
                                         rhs=ones_s[:, :BL], start=(t == 0),
                                         stop=True)
            if t == 3:
                # emb-half of the decoder input kernel in a GRU PE gap
                # (deckB has landed; copying to SBUF frees its slot so the
                # in-order DMA queue can start deckA's transfer)
                pxB = ps_xmd.tile([128, 3, C, BL], F32, tag="pxB")
                for g in range(3):
                    for c in range(C):
                        col0 = g * D + c * 128
                        for kp in range(CP):
                            nc.tensor.matmul(
                                out=pxB[:, g, c],
                                lhsT=deckB_s[:, 2 * kp:2 * kp + 2,
                                             col0:col0 + 128],
                                rhs=xembT_8[:, 2 * kp:2 * kp + 2],
                                start=(kp == 0), stop=False, perf_mode=DR)
                        nc.tensor.matmul(out=pxB[:, g, c],
                                         lhsT=dbx_s[:, col0:col0 + 128],
                                         rhs=ones_s[:, :BL], start=False,
                                         stop=True)
            if t == 4:
                nc.vector.tensor_copy(out=xmdB_sb[:], in_=pxB[:])
            # sigmoid(x) == (tanh(x/2)+1)/2; affine parts folded on host
            mask_bc = _bcast_mid(mask_s[:, t, :], C)
            tz = gtmp.tile([128, C, BL], F32, tag="tz")
            tr = gtmp.tile([128, C, BL], F32, tag="tr")
            nc.scalar.activation(out=tr[:], in_=phm[:, 1], func=AF.Tanh,
                                 scale=0.5)
            nc.scalar.activation(out=tz[:], in_=phm[:, 0], func=AF.Tanh,
                                 scale=0.5)
            # rhh = (tanh_r + 1) * hh == 2*r*hh; xg-h host-doubled
            # rhh = (tanh_r + 1) * hh; at t==0 phm[2] is the h-bias only
            rhh = gtmp.tile([128, C, BL], F32, tag="rhh")
            nc.vector.scalar_tensor_tensor(out=rhh[:], in0=tr[:],
                                           scalar=1.0, in1=phm[:, 2],
                                           op0=OP.add, op1=OP.mult)
            cin = gtmp.tile([128, C, BL], F32, tag="cin")
            nc.vector.tensor_tensor(out=cin[:], in0=xg[2][:, :, :, t],
                                    in1=rhh[:], op=OP.add)
            # zcm = (1-z)*mask == (tanh_z - 1) * (-0.5*mask)
            zcm = gtmp.tile([128, C, BL], F32, tag="zcm")
            nc.vector.scalar_tensor_tensor(out=zcm[:], in0=tz[:], scalar=-1.0,
                                           in1=mask_bc, op0=OP.add,
                                           op1=OP.mult)
            h_f2 = hstate.tile([128, C, BL], F32, tag="h_f")
            if t > 0:
                hz1 = gtmp.tile([128, C, BL], F32, tag="hz1")
                nc.vector.tensor_tensor(out=hz1[:], in0=h_f[:], in1=zcm[:],
                                        op=OP.mult)
                hm1 = gtmp.tile([128, C, BL], F32, tag="hm1")
                nc.vector.tensor_tensor(out=hm1[:], in0=h_f[:], in1=hz1[:],
                                        op=OP.subtract)
            cand = gtmp.tile([128, C, BL], F32, tag="cand")
            nc.scalar.activation(out=cand[:], in_=cin[:], func=AF.Tanh,
                                 scale=0.5)
            if t == 0:
                nc.vector.tensor_tensor(out=seq8[:, :, :, 0], in0=cand[:],
                                        in1=zcm[:], op=OP.mult)
                nc.vector.tensor_tensor(out=h_f2[:], in0=cand[:], in1=zcm[:],
                                        op=OP.mult)
            else:
                t2 = gtmp.tile([128, C, BL], F32, tag="t2")
                nc.vector.tensor_tensor(out=t2[:], in0=cand[:], in1=zcm[:],
                                        op=OP.mult)
                nc.vector.tensor_tensor(out=seq8[:, :, :, t], in0=hm1[:],
                                        in1=t2[:], op=OP.add)
                nc.vector.tensor_tensor(out=h_f2[:], in0=hm1[:], in1=t2[:],
                                        op=OP.add)
            h_f = h_f2
    if DEBUG:
        dbg_seq = pers.tile([128, C, BL, T], F32, tag="dbg_seq")
        nc.vector.tensor_copy(out=dbg_seq[:], in_=seq8[:])
        nc.sync.dma_start(out=dbg["seq"], in_=dbg_seq[:])

    # =================== STAGE 3: utterance attention ===================
    seq8v = seq8[:].rearrange("p c b t -> p c (b t)")
    with tc.tile_pool(name="ps_su", bufs=2, space="PSUM") as ps_su, \
         tc.tile_pool(name="ps_scu", bufs=1, space="PSUM") as ps_scu, \
         tc.tile_pool(name="s3tmp", bufs=2) as s3tmp:
        su8 = s3tmp.tile([128, C, BL, T], FP8, tag="su8")
        for half in range(2):
            psu = ps_su.tile([128, 4, BL, T], F32, tag="psu")
            for mm in range(4):
                m = half * 4 + mm
                for kp in range(CP):
                    nc.tensor.matmul(
                        out=psu[:, mm],
                        lhsT=w1u_s[:, 2 * kp:2 * kp + 2,
                                   m * 128:(m + 1) * 128],
                        rhs=seq8v[:, 2 * kp:2 * kp + 2],
                        start=(kp == 0), stop=(kp == CP - 1), perf_mode=DR)
            qn = s3tmp.tile([128, 4, BL, T], F32, tag="qn")
            nc.vector.tensor_tensor(
                out=qn[:], in0=psu[:],
                in1=_bcast_last(qu_s[:, half * 4:half * 4 + 4], T), op=OP.add)
            nc.scalar.activation(out=su8[:, half * 4:half * 4 + 4],
                                 in_=qn[:], func=AF.Tanh)
        # hm_dec = hidden @ dec_rec_kernel on PE under the stage-3
        # softmax's DVE/ACT shadow (decrk pairs have landed by now)
        for k in range(C):
            dk = decrk_tiles[k // 2][:, k % 2]
            phmd_k = ps_hmd.tile([128, 3, C, BL], F32, tag="phmd_k")
            for g in range(3):
                for c in range(C):
                    col0 = g * D + c * 128
                    nc.tensor.matmul(out=phmd_k[:, g, c],
                                     lhsT=dk[:, col0:col0 + 128],
                                     rhs=hidT_b[:, k], start=True,
                                     stop=(k != C - 1 or g != 2))
                    if k == C - 1 and g == 2:
                        nc.tensor.matmul(
                            out=phmd_k[:, g, c],
                            lhsT=db1h_s[:, c * 128:(c + 1) * 128],
                            rhs=ones_s[:, :BL], start=False, stop=True)
            if k == 0:
                nc.vector.tensor_copy(out=hmd_sb[:], in_=phmd_k[:])
            else:
                nc.vector.tensor_tensor(out=hmd_sb[:], in0=hmd_sb[:],
                                        in1=phmd_k[:], op=OP.add)
        pscu = ps_scu.tile([128, BL, T], F32)
        for cp in range(CP):
            nc.tensor.matmul(out=pscu[:], lhsT=vu_s[:, 2 * cp:2 * cp + 2],
                             rhs=su8[:, 2 * cp:2 * cp + 2],
                             start=(cp == 0), stop=(cp == CP - 1),
                             perf_mode=DR)
        eu = s3tmp.tile([128, BL, T], BF16, tag="eu")
        nc.scalar.activation(out=eu[:], in_=pscu[:], func=AF.Exp)
        rsu = s3tmp.tile([128, BL], F32, tag="rsu")
        nc.vector.reduce_sum(out=rsu[:], in_=eu[:], axis=AX.X)
        rcu = s3tmp.tile([128, BL], F32, tag="rcu")
        nc.vector.reciprocal(out=rcu[:], in_=rsu[:])
        pru = s3tmp.tile([128, C, BL, T], BF16, tag="pru")
        nc.vector.tensor_tensor(out=pru[:], in0=seq8[:],
                                in1=_bcast_mid(eu[:], C), op=OP.mult)
        redu = s3tmp.tile([128, C, BL], F32, tag="redu")
        nc.vector.reduce_sum(out=redu[:], in_=pru[:], axis=AX.X)
        nc.vector.tensor_tensor(out=ctxv8[:], in0=redu[:],
                                in1=_bcast_mid(rcu[:], C), op=OP.mult)
    if DEBUG:
        dbg_cv = pers.tile([128, C, BL], F32, tag="dbg_cv")
        nc.vector.tensor_copy(out=dbg_cv[:], in_=ctxv8[:])
        nc.sync.dma_start(out=dbg["ctxv"], in_=dbg_cv[:])

    # =================== STAGE 4: decoder GRU step ===================
    with tc.tile_pool(name="s4tmp", bufs=1) as s4tmp:
        nc.vector.tensor_tensor(out=bh_sb[:], in0=hmd_sb[:], in1=xmdB_sb[:],
                                op=OP.add)
        if DEBUG:
            nc.sync.dma_start(out=dbg["hmd"], in_=hmd_sb[:])

        # ctxv-half (deckA)
        pxA = ps_xmd.tile([128, 3, C, BL], F32, tag="pxA")
        for g in range(3):
            for c in range(C):
                col0 = g * D + c * 128
                for kp in range(CP):
                    nc.tensor.matmul(
                        out=pxA[:, g, c],
                        lhsT=deckA_s[:, 2 * kp:2 * kp + 2, col0:col0 + 128],
                        rhs=ctxv8[:, 2 * kp:2 * kp + 2],
                        start=(kp == 0), stop=(kp == CP - 1), perf_mode=DR)
        if DEBUG:
            dbg_xm = s4tmp.tile([128, 3, C, BL], F32, tag="dbg_xm")
            nc.vector.tensor_tensor(out=dbg_xm[:], in0=xmdB_sb[:],
                                    in1=pxA[:], op=OP.add)
            nc.sync.dma_start(out=dbg["xmd"], in_=dbg_xm[:])

        tz = s4tmp.tile([128, C, BL], F32, tag="tz4")
        tr = s4tmp.tile([128, C, BL], F32, tag="tr4")
        rin = s4tmp.tile([128, C, BL], F32, tag="rin4")
        nc.vector.tensor_tensor(out=rin[:], in0=pxA[:, 1], in1=bh_sb[:, 1],
                                op=OP.add)
        nc.scalar.activation(out=tr[:], in_=rin[:], func=AF.Tanh, scale=0.5)
        zin = s4tmp.tile([128, C, BL], F32, tag="zin4")
        nc.vector.tensor_tensor(out=zin[:], in0=pxA[:, 0], in1=bh_sb[:, 0],
                                op=OP.add)
        nc.scalar.activation(out=tz[:], in_=zin[:], func=AF.Tanh, scale=0.5)
        # candidate: cin/2 = xh + r*hh with xh = xA_h + xB_h + b0_h (host-
        # doubled cols/bias), hh = hmd_h + b1_h (plain). bh[2] mixes hmd_h
        # into the x-side, so use pxB[2] and hmd_sb[2] directly here.
        rhh = s4tmp.tile([128, C, BL], F32, tag="rhh4")
        nc.vector.scalar_tensor_tensor(out=rhh[:], in0=tr[:], scalar=1.0,
                                       in1=hmd_sb[:, 2], op0=OP.add,
                                       op1=OP.mult)
        xh = s4tmp.tile([128, C, BL], F32, tag="xh4")
        nc.vector.tensor_tensor(out=xh[:], in0=pxA[:, 2], in1=xmdB_sb[:, 2],
                                op=OP.add)
        cin = s4tmp.tile([128, C, BL], F32, tag="cin4")
        nc.vector.tensor_tensor(out=cin[:], in0=xh[:], in1=rhh[:], op=OP.add)
        cand = s4tmp.tile([128, C, BL], F32, tag="cand4")
        nc.scalar.activation(out=cand[:], in_=cin[:], func=AF.Tanh, scale=0.5)
        zcm = s4tmp.tile([128, C, BL], F32, tag="zcm4")
        nc.vector.tensor_scalar(out=zcm[:], in0=tz[:], scalar1=-1.0,
                                scalar2=-0.5, op0=OP.add, op1=OP.mult)
        d1 = s4tmp.tile([128, C, BL], F32, tag="d14")
        nc.vector.tensor_tensor(out=d1[:], in0=cand[:], in1=hidT_f[:],
                                op=OP.subtract)
        d2 = s4tmp.tile([128, C, BL], F32, tag="d24")
        nc.vector.tensor_tensor(out=d2[:], in0=d1[:], in1=zcm[:], op=OP.mult)
        stT = s4tmp.tile([128, C, BL], F32, tag="stT")
        nc.vector.tensor_tensor(out=stT[:], in0=hidT_f[:], in1=d2[:],
                                op=OP.add)
        nc.sync.dma_start(out=ins["out"], in_=stT[:])

    es.close()


# ---------------------------------------------------------------------------
# Host side
# ---------------------------------------------------------------------------

_NC_CACHE = {}


def _get_nc():
    key = ("prog_v2", DEBUG)
    if key not in _NC_CACHE:
        _NC_CACHE[key] = build()
    return _NC_CACHE[key]


def _f8(a):
    return np.ascontiguousarray(np.asarray(a, np.float32)
                                .astype(ml_dtypes.float8_e4m3fn))


def _bf(a):
    return np.ascontiguousarray(np.asarray(a, np.float32)
                                .astype(ml_dtypes.bfloat16))


def _f32(a):
    return np.ascontiguousarray(np.asarray(a, np.float32))


def _chunked_T(w):
    """[D_in, N] -> [128, D_in//128, N]: row-chunked per-k lhsT tiles."""
    d_in, n = w.shape
    return np.ascontiguousarray(w.reshape(d_in // 128, 128, n)
                                .transpose(1, 0, 2))


def prepare_in_maps(inputs):
    x = np.asarray(inputs["x"]).astype(np.int64).reshape(B)
    hidden = _f32(inputs["hidden"])
    enc = _f32(inputs["encoder_outputs"])          # [64, 10, 50, 1024]
    maskf = np.asarray(inputs["context_mask"]).astype(np.float32)
    emb = np.asarray(inputs["embed_table"])

    x_emb = emb[x].astype(np.float32)

    def tmajor(a2d):  # [B, D] -> [128, C, B]
        return np.ascontiguousarray(
            a2d.T.reshape(C, 128, a2d.shape[0]).transpose(1, 0, 2))

    def dbl_h(w):
        w = np.array(w, np.float32, copy=True)
        w[:, 2 * D:] *= 2.0
        return w

    w1w = _f8(_chunked_T(np.asarray(inputs["w1_word"], np.float32)))
    w2w = _f8(_chunked_T(np.asarray(inputs["w2_word"], np.float32)))
    w1u = _f8(_chunked_T(np.asarray(inputs["w1_utt"], np.float32)))
    w2u = _f8(_chunked_T(np.asarray(inputs["w2_utt"], np.float32)))
    ctxk = _f8(_chunked_T(dbl_h(np.asarray(inputs["ctx_kernel"], np.float32))))
    ctxrk = _f8(_chunked_T(np.asarray(inputs["ctx_rec_kernel"], np.float32)))
    deck_full = dbl_h(np.asarray(inputs["dec_kernel"], np.float32))
    deckA = _f8(_chunked_T(deck_full[:D]))
    deckB = _f8(_chunked_T(deck_full[D:]))
    decrk = _bf(np.asarray(inputs["dec_rec_kernel"], np.float32)
                .reshape(C, 128, G3))

    def vrep(v):
        vc = np.asarray(v, np.float32).reshape(C, 128).T
        return _f8(np.broadcast_to(vc[:, :, None], (128, C, 128)))

    vw = vrep(inputs["v_word"])
    vu = vrep(inputs["v_utt"])

    def mchunk(v):
        return _f32(np.asarray(v, np.float32).reshape(C, 128).T)

    qb_w = mchunk(np.asarray(inputs["b1_word"], np.float32)
                  + np.asarray(inputs["b2_word"], np.float32))
    qb_u = mchunk(np.asarray(inputs["b1_utt"], np.float32)
                  + np.asarray(inputs["b2_utt"], np.float32))

    cbias = np.asarray(inputs["ctx_bias"], np.float32)
    dbias = np.asarray(inputs["dec_bias"], np.float32)

    def gate_bias_row(bias2):
        return np.concatenate([
            bias2[0, :D] + bias2[1, :D],
            bias2[0, D:2 * D] + bias2[1, D:2 * D],
            2.0 * bias2[0, 2 * D:],
        ]).reshape(1, G3)

    cbx = _bf(gate_bias_row(cbias))
    dbx = _bf(gate_bias_row(dbias))
    cb1h = _bf(cbias[1, 2 * D:].reshape(1, D))
    db1h = _bf(dbias[1, 2 * D:].reshape(1, D))

    ones_b = _bf(np.ones((1, BL * T), np.float32))

    enc_r = enc.reshape(B, R, D)

    in_maps = []
    for core in range(NCORES):
        sl = slice(core * BL, (core + 1) * BL)
        enc_t = np.ascontiguousarray(
            enc_r[sl].transpose(0, 2, 1)
            .reshape(BL, C, 128, R)
            .transpose(0, 2, 1, 3))
        hid_c = hidden[sl]
        mask_t = np.ascontiguousarray(
            np.broadcast_to(-0.5 * maskf[sl].T[None, :, :], (128, T, BL)))
        in_maps.append({
            "enc_t": _f8(enc_t),
            "hidT_f": _f32(tmajor(hid_c)),
            "hidT_b": _bf(tmajor(hid_c)),
            "hidT_8": _f8(tmajor(hid_c)),
            "xembT_8": _f8(tmajor(x_emb[sl])),
            "w1w": w1w, "w2w": w2w, "vw_rep": vw,
            "w1u": w1u, "w2u": w2u, "vu_rep": vu,
            "ctxk": ctxk, "ctxrk": ctxrk,
            "deckA": deckA, "deckB": deckB, "decrk": decrk,
            "qb_w": qb_w, "qb_u": qb_u,
            "cbx_row": cbx, "cb1h_b": cb1h,
            "dbx_row": dbx, "db1h_b": db1h,
            "mask_t": _f32(mask_t),
            "ones_b": ones_b,
        })
    return in_maps


def run(inputs):
    nc = _get_nc()
    in_maps = prepare_in_maps(inputs)
    res = run_bass_kernel_spmd(nc, in_maps, list(range(NCORES)))
    # out per core: [128, C, BL] feature-major; host transposes to [BL, D]
    parts = []
    for c in range(NCORES):
        o = np.asarray(res.results[c]["out"])           # [128, C, BL]
        parts.append(o.transpose(2, 1, 0).reshape(BL, D))
    out = np.concatenate(parts, axis=0)
    return np.ascontiguousarray(out.astype(np.float32)), res


def kernel(**inputs):
    out, _ = run(inputs)
    return out, out
